# revision 3
# baseline (speedup 1.0000x reference)
"""Trainium2 Bass kernel for nn_CNNtoGraph_77936476553433 (8-core k-parallel).

The GNN collapses algebraically: per sample b
    out[b] = x[b] @ W2 + bias,   x[b] = [(1/30) sum_u s[b,u] pf[b,u,:],
                                         (1/6)  sum_u        pf[b,u,:]]  (R^4096)
    W2 = fc_w @ cls_w  (4096x200),  bias = fc_b @ cls_w + cls_b
with s[b,u] = sum_v w[b,u,v] the edge-weight row sums from cdds box centers.

Sharding: the CONTRACTION dim k (=2*D) is split 8 ways. Each core streams the
full batch but only its 256-column d-slice of part_feats (12.6 MB), computes
s for all samples (stage 0, pipelined in 3 column-chunks), forms its 512-row
slice of xT (stage 1), computes its W2 k-slice from a host-pretransposed bf16
fc_w slice (no PE transposes, no AllGather), and accumulates partial outputs
out_part[b, c] over its k-slice (stage 2, sample-major PSUM so partials DMA
straight out). A 2-chunk bf16 ReduceScatter sums the 8 partials and hands
each core two 128-sample shards; bias is added once per shard afterwards.
"""
import sys
sys.path.insert(0, '/opt/trn_rl_repo')
import numpy as np
import ml_dtypes
import concourse.bass as bass
import concourse.bacc as bacc
import concourse.tile as tile
import concourse.mybir as mybir
from concourse import bass_utils

N_CORES = 8
B_FULL = 2048

F32 = mybir.dt.float32
F32R = mybir.dt.float32r
BF16 = mybir.dt.bfloat16
ALU = mybir.AluOpType
ACTF = mybir.ActivationFunctionType
ALPHA = 0.015

D, H, C, NN = 2048, 1024, 200, 6
CP = 256                    # C padded to a 512-byte bf16 line
RT, TS = 126, 21            # rows per sample-tile, samples per sample-tile
DL = D // N_CORES           # d-columns per core (256)
NKT = (2 * DL) // 128       # 4 k-tiles per core
NHT = H // 128              # 8 h-tiles
JC = 33                     # stage-0 pipeline chunk (NJS = 3*JC)


def ap_of(ap, offset, pattern):
    return bass.AP(ap.tensor, offset, pattern)


def ins_bcast(ap, idx, n):
    """Insert a broadcast (step-0) dim into an AP at position idx."""
    a = [list(d) for d in ap.ap]
    a.insert(idx, [0, n])
    return bass.AP(ap.tensor, ap.offset, a)


def bcast_last(ap, n):
    """Replace a singleton last dim with a step-0 broadcast of size n."""
    a = [list(d) for d in ap.ap]
    assert a[-1][1] == 1, a
    return bass.AP(ap.tensor, ap.offset, a[:-1] + [[0, n]])


def build_nc(B_loc=B_FULL, n_cores=8, **_):
    B = B_FULL                          # full batch on every core
    NJ = -(-B // TS)                    # 98 sample-tiles
    b0s = [TS * j for j in range(NJ - 1)] + [B - TS]
    LO = NJ * TS - B                    # overlap of last stile (10)
    NL = TS - LO                        # new samples in last stile (11)
    NJS = NJ + (1 if LO else 0)         # stage-0 columns (99)
    NBT = B // 128                      # 16 output b-tiles
    assert NJS == 3 * JC
    # pf super-tiles over tiles 0..NJ-2; tile NJ-1 (the overlap tail) is solo
    STJ = 7
    sts = []
    j0 = 0
    while j0 < NJ - 1:
        J = min(STJ, NJ - 1 - j0)
        sts.append((j0, J))
        j0 += J

    nc = bacc.Bacc("TRN2", target_bir_lowering=False, debug=False,
                   enable_asserts=True, num_devices=n_cores)
    pf = nc.dram_tensor("pf", [B * NN, DL], F32, kind="ExternalInput").ap()
    # box coords pre-gathered on host into stage-0's partition layout:
    # cdds4[p=(t,u), j, 0:4] = cdds[b0s[j] + t, 1+6*u : 5+6*u]
    cdds4 = nc.dram_tensor("cdds4", [RT, NJS * 4], F32,
                           kind="ExternalInput").ap()
    fcwT = nc.dram_tensor("fcwT", [H, 2 * DL], BF16, kind="ExternalInput").ap()
    clsw = nc.dram_tensor("clsw", [H, CP], BF16, kind="ExternalInput").ap()
    biasr = nc.dram_tensor("biasr", [1, C], F32, kind="ExternalInput").ap()
    # small f32 stage-0 constants in one tensor/DMA: [sel6 | mask_c | ma | mp]
    NCC = 6 + 6 + TS + TS
    consts = nc.dram_tensor("consts", [RT, NCC], F32, kind="ExternalInput").ap()
    gsum = nc.dram_tensor("gsum", [RT, RT], F32, kind="ExternalInput").ap()
    out = nc.dram_tensor("out", [2 * 128, C], F32, kind="ExternalOutput").ap()

    with tile.TileContext(nc) as tc:
        with tc.tile_pool(name="persist", bufs=1) as pp, \
             tc.tile_pool(name="dram", bufs=1, space="DRAM") as dp:

            # ---------------- persistent SBUF ----------------
            xT = pp.tile([128, NKT * B], BF16)           # stage-2 lhsT source
            wall = pp.tile([RT, NJS * 42], F32)          # stage-1 rhs (block diag)
            fcw_sb = pp.tile([128, NHT * 2 * DL], BF16)  # [h%128, ht, k]
            clsw_sb = pp.tile([128, NHT * CP], BF16)     # [h%128, ht, c]
            w2b = pp.tile([128, NKT * CP], BF16)         # [k%128, kt, c]
            bias_sb = pp.tile([1, C], F32)
            bias_bc = pp.tile([128, C], F32)
            ones_sb = pp.tile([1, 128], F32)
            scr_act = pp.tile([1, 8], F32)
            c_all = pp.tile([RT, NCC], F32)
            c_sel6 = c_all[:, 0:6]
            c_maskc = c_all[:, 6:12]
            c_ma21 = c_all[:, 12:12 + TS]
            c_mp21 = c_all[:, 12 + TS:NCC]
            # stage-0 working set
            own4 = pp.tile([RT, NJS * 4], F32)
            sxy = pp.tile([RT, NJS * 2], F32)
            rhs_all = pp.tile([RT, NJS * 12], F32)
            all_xy = pp.tile([RT, NJS * 12], F32)
            dall = pp.tile([RT, NJS * 6], F32)           # dx, d2, dist, em_minus
            dall2 = pp.tile([RT, NJS * 6], F32)          # dy, then relu scratch
            em = pp.tile([RT, NJS * 6], F32)
            esum = pp.tile([RT, NJS], F32)
            mean_sb = pp.tile([RT, NJS], F32)
            s_col = pp.tile([RT, NJS], F32)

            b_in = dp.tile([B, CP], BF16)
            b_out = dp.tile([256, CP], BF16)

            # -------- DMAs: consts + cdds4 + weights ahead of the pf stream
            c_gsum = pp.tile([RT, RT], F32)
            with tc.high_priority():
                nc.scalar.dma_start(c_all[:], consts)
                nc.scalar.dma_start(c_gsum[:], gsum)
                nc.scalar.dma_start(bias_sb[:], biasr)
                nc.sync.dma_start(own4[:], cdds4)
                nc.sync.dma_start(
                    fcw_sb[:].rearrange("p (ht k) -> p ht k", k=2 * DL),
                    fcwT.rearrange("(ht p) k -> p ht k", p=128))
                nc.sync.dma_start(
                    clsw_sb[:].rearrange("p (ht c) -> p ht c", c=CP),
                    clsw.rearrange("(ht p) c -> p ht c", p=128))
            # ACT table prewarm for Sqrt (Exp's load stays on the chain once)
            nc.gpsimd.memset(scr_act[:], 1.0)
            nc.scalar.activation(scr_act[:], scr_act[:], ACTF.Sqrt, scale=1.0)
            nc.gpsimd.memset(ones_sb[:], 1.0)
            # wall p-part is constant: wall[:, j, 21:42] = mp21 (all j)
            wv = wall[:].rearrange("p (j f) -> p j f", f=42)
            nc.gpsimd.tensor_copy(
                ap_of(wall[:], 21, [[NJS * 42, RT], [42, NJS], [1, TS]]),
                ins_bcast(c_mp21, 1, NJS))

            # ---------------- stage 0: edge weights (3 chunks) ------------
            o4 = own4[:].rearrange("p (j f) -> p j f", f=4)
            sx2 = sxy[:].rearrange("p (j f) -> p j f", f=2)
            r12 = rhs_all[:].rearrange("p (j f) -> p j f", f=12)
            a12 = all_xy[:].rearrange("p (j f) -> p j f", f=12)
            d6 = dall[:].rearrange("p (j f) -> p j f", f=6)
            e6 = dall2[:].rearrange("p (j f) -> p j f", f=6)
            m6 = em[:].rearrange("p (j f) -> p j f", f=6)
            CH = [slice(q * JC, (q + 1) * JC) for q in range(3)]

            def sel_(q):
                return ins_bcast(c_sel6, 1, JC)

            for q, s in enumerate(CH):
                nc.vector.tensor_add(sx2[:, s, 1:2], o4[:, s, 0:1],
                                     o4[:, s, 2:3])
                nc.vector.tensor_add(sx2[:, s, 0:1], o4[:, s, 1:2],
                                     o4[:, s, 3:4])
                nc.vector.tensor_mul(r12[:, s, 0:6], sel_(q),
                                     bcast_last(sx2[:, s, 0:1], 6))
                nc.vector.tensor_mul(r12[:, s, 6:12], sel_(q),
                                     bcast_last(sx2[:, s, 1:2], 6))

            with tc.tile_pool(name="ps0", bufs=1, space="PSUM") as ps0:
                for q, s in enumerate(CH):
                    gch = ps0.tile([RT, JC * 12], F32, tag="gch", bufs=2)
                    nc.tensor.matmul(gch[:], c_gsum[:],
                                     rhs_all[:, q * JC * 12:(q + 1) * JC * 12],
                                     start=True, stop=True)
                    nc.vector.tensor_copy(
                        all_xy[:, q * JC * 12:(q + 1) * JC * 12], gch[:])
                for q, s in enumerate(CH):
                    sx_b = bcast_last(sx2[:, s, 0:1], 6)
                    sy_b = bcast_last(sx2[:, s, 1:2], 6)
                    nc.gpsimd.tensor_sub(e6[:, s, :], sy_b, a12[:, s, 6:12])
                    nc.gpsimd.tensor_mul(e6[:, s, :], e6[:, s, :], e6[:, s, :])
                    nc.vector.tensor_sub(d6[:, s, :], sx_b, a12[:, s, 0:6])
                    nc.vector.tensor_mul(d6[:, s, :], d6[:, s, :], d6[:, s, :])
                    nc.vector.tensor_add(d6[:, s, :], d6[:, s, :], e6[:, s, :])
                # single sqrt + single exp (chunking would thrash the ACT
                # function table: 1.28us reload per Sqrt<->Exp switch)
                nc.scalar.activation(dall[:], dall[:], ACTF.Sqrt, scale=0.25)
                nc.scalar.activation(dall[:], dall[:], ACTF.Exp, scale=-ALPHA)
                for q, s in enumerate(CH):
                    # em = exp * mask ; esum = sum_v em
                    nc.vector.tensor_mul(m6[:, s, :], d6[:, s, :],
                                         ins_bcast(c_maskc, 1, JC))
                    nc.vector.tensor_reduce(
                        esum[:, s], m6[:, s, :], mybir.AxisListType.X,
                        ALU.add)
                for q, s in enumerate(CH):
                    mps = ps0.tile([RT, JC], F32, tag="mps", bufs=2)
                    nc.tensor.matmul(mps[:], c_gsum[:], esum[:, s],
                                     start=True, stop=True)
                    nc.vector.tensor_copy(mean_sb[:, s], mps[:])
                for q, s in enumerate(CH):
                    # em_minus = em - mean/30 ; s' = sum_v relu(em_minus)
                    # (the 0.8 = SCALE/30 factor is folded into ma21)
                    nc.vector.scalar_tensor_tensor(
                        d6[:, s, :], ins_bcast(mean_sb[:, s], 2, 6),
                        -1.0 / 30.0, m6[:, s, :], op0=ALU.mult, op1=ALU.add)
                    nc.vector.tensor_relu(e6[:, s, :], d6[:, s, :])
                    nc.vector.tensor_reduce(s_col[:, s], e6[:, s, :],
                                            mybir.AxisListType.X, ALU.add)
                    # wall a-part: wall[:, j, 0:21] = (0.8*ma21) * s'[:, j]
                    nc.vector.tensor_mul(
                        ap_of(wall[:], q * JC * 42,
                              [[NJS * 42, RT], [42, JC], [1, TS]]),
                        ins_bcast(c_ma21, 1, JC),
                        ins_bcast(s_col[:, s], 2, TS))

            # ---------------- W2 k-slice + bias broadcast ----------------
            fS = fcw_sb[:].rearrange("p (ht k) -> p ht k", k=2 * DL)
            cS = clsw_sb[:].rearrange("p (ht c) -> p ht c", c=CP)
            w2v = w2b[:].rearrange("p (kt c) -> p kt c", c=CP)
            with tc.tile_pool(name="psw", bufs=1, space="PSUM") as psw:
                for kt in range(NKT):
                    wps = psw.tile([128, CP], F32, tag="wps", bufs=2)
                    for ht in range(NHT):
                        nc.tensor.matmul(
                            wps[:], fS[:, ht, kt * 128:(kt + 1) * 128],
                            cS[:, ht, :],
                            start=(ht == 0), stop=(ht == NHT - 1))
                    nc.vector.tensor_copy(w2v[:, kt, :], wps[:])
                bps = psw.tile([128, C], F32, tag="bps")
                nc.tensor.matmul(bps[:], ones_sb[:], bias_sb[:], start=True,
                                 stop=True)
                nc.vector.tensor_copy(bias_bc[:], bps[:])

            # ---------------- stage 1 + interleaved stage 2 ----------------
            xv = xT[:].rearrange("p (kt b) -> p kt b", b=B)
            lo, nl = LO, NL
            state = {"bt": 0, "ev": 0, "pt": None}
            with tc.tile_pool(name="pfp", bufs=5) as pfp, \
                 tc.tile_pool(name="ps1", bufs=1, space="PSUM") as ps1, \
                 tc.tile_pool(name="ps2", bufs=1, space="PSUM") as ps2, \
                 tc.tile_pool(name="ptp", bufs=8) as ptp:

                def do_btile(t):
                    ops = ps2.tile([128, CP], F32, tag="ops", bufs=2)
                    for kt in range(NKT):
                        nc.tensor.matmul(
                            ops[:],
                            ap_of(xT[:], kt * B + 128 * t,
                                  [[NKT * B, 128], [1, 128]]),
                            w2v[:, kt, :],
                            start=(kt == 0), stop=(kt == NKT - 1))
                    if t % 2 == 0:
                        state["pt"] = ptp.tile([128, 2 * CP], BF16, tag="pt",
                                               bufs=4, name="ptb")
                    pt = state["pt"]
                    nc.scalar.copy(pt[:, (t % 2) * CP:(t % 2 + 1) * CP], ops[:])
                    if t % 2 == 1:
                        # rows 128(t-1) .. 128(t+1) of b_in in one DMA
                        nc.gpsimd.dma_start(
                            ap_of(b_in.opt(), (t - 1) * 128 * CP,
                                  [[CP, 128], [128 * CP, 2], [1, CP]]),
                            ap_of(pt[:], 0, [[2 * CP, 128], [CP, 2], [1, CP]]))
                    if t == NBT - 1:
                        nc.gpsimd.collective_compute(
                            "ReduceScatter", ALU.add,
                            replica_groups=[list(range(n_cores))],
                            ins=[b_in.opt()], outs=[b_out.opt()])

                def post():
                    # shard rows (h*128+p) -> SBUF [p, h, c]; add bias; store
                    pb = pp.tile([128, 2 * CP], BF16, name="pb")
                    pfq = pp.tile([128, 2 * C], F32, name="pfq")
                    nc.sync.dma_start(
                        pb[:].rearrange("p (h c) -> p h c", c=CP),
                        b_out.opt().rearrange("(h p) c -> p h c", p=128))
                    nc.vector.tensor_add(
                        pfq[:].rearrange("p (h c) -> p h c", c=C),
                        ap_of(pb[:], 0, [[2 * CP, 128], [CP, 2], [1, C]]),
                        ins_bcast(bias_bc[:], 1, 2))
                    nc.sync.dma_start(
                        out.rearrange("(h p) c -> p h c", p=128),
                        pfq[:].rearrange("p (h c) -> p h c", c=C))

                def do_tile(j, pft_ap):
                    last = j == NJ - 1 and lo > 0
                    rt = 6 * nl if last else RT
                    ns = nl if last else TS
                    if last:
                        rhs_w = ap_of(wall[:], (NJS - 1) * 42,
                                      [[NJS * 42, rt], [21, 2], [1, ns]])
                    else:
                        rhs_w = wv[:, j, :]
                    psA = ps1.tile([128, 84], F32, tag="psA", bufs=4)
                    for db in range(2):
                        nc.tensor.matmul(
                            psA[:, db * 2 * ns:(db + 1) * 2 * ns],
                            pft_ap[0:rt, db * 128:(db + 1) * 128],
                            rhs_w, start=True, stop=True)
                    c0 = b0s[j] + lo if last else b0s[j]
                    # one 4D copy: psA[p, db, h, s] -> xT[p, (2h+db)*B + c0+s]
                    src = ap_of(psA[:], 0,
                                [[84, 128], [ns, 2], [2 * ns, 2], [1, ns]])
                    dst = ap_of(xT[:], c0,
                                [[NKT * B, 128], [2 * B, 2], [B, 2], [1, ns]])
                    if state["ev"] % 2 == 0:
                        nc.vector.tensor_copy(dst, src)
                    else:
                        nc.scalar.copy(dst, src)
                    state["ev"] += 1
                    if last:
                        return   # tail runs first; it must not advance bt
                    cov = TS * (j + 1)   # samples covered by regular tiles
                    while state["bt"] < NBT and (
                            128 * (state["bt"] + 1) <= cov
                            or (state["bt"] == NBT - 1 and cov >= B - NL)):
                        do_btile(state["bt"])
                        state["bt"] += 1

                pf_tl = pfp.tile([6 * NL, DL], F32, tag="pftail", bufs=1)
                nc.sync.dma_start(pf_tl[:], pf[(B - NL) * 6:B * 6, :])
                do_tile(NJ - 1, pf_tl[:])
                for si, (j0, J) in enumerate(sts):
                    pf_st = pfp.tile([RT, STJ * DL], F32, tag="pf", bufs=5)
                    nc.sync.dma_start(
                        pf_st[:, 0:J * DL].rearrange(
                            "p (jj d) -> p jj d", d=DL),
                        ap_of(pf, j0 * RT * DL,
                              [[DL, RT], [RT * DL, J], [1, DL]]))
                    for jj in range(J):
                        do_tile(j0 + jj, pf_st[:, jj * DL:(jj + 1) * DL])
                while state["bt"] < NBT:
                    do_btile(state["bt"])
                    state["bt"] += 1
                post()
    nc.compile()
    return nc


def make_host_inputs(part_feats, cdds, fc_w, fc_b, cls_w, cls_b, n_cores=8):
    """Shard + prepare per-core in_maps from full inputs."""
    B = part_feats.shape[0]
    p = np.arange(RT)
    maskc = (p[:, None] % 6 != np.arange(6)[None, :]).astype(np.float32)
    sel = (p[:, None] % 6 == np.arange(6)[None, :]).astype(np.float32)
    gs = (p[:, None] // 6 == p[None, :] // 6).astype(np.float32)
    ma = np.zeros((RT, TS), np.float32)
    ma[p, p // 6] = 0.8                  # SCALE/30 folded in
    mp = np.zeros((RT, TS), np.float32)
    mp[p, p // 6] = 1.0 / 6.0
    bias = (fc_b @ cls_w + cls_b).reshape(1, C).astype(np.float32)
    # cdds4[p=(t,u), j, 0:4] = cdds[b0s[j] + t, 1+6u : 5+6u]; tail column
    # (j = NJ) = the last NL samples at partitions 0:6*NL, zero-padded.
    NJ = -(-B // TS)
    LO = NJ * TS - B
    NL = TS - LO
    NJS = NJ + (1 if LO else 0)
    b0s = np.array([TS * j for j in range(NJ - 1)] + [B - TS])
    cd = cdds.reshape(B, 6, 6)[:, :, 1:5]                # (B, u, 4)
    t_i, u_i = p // 6, p % 6
    cdds4 = np.zeros((RT, NJS, 4), np.float32)
    cdds4[:, 0:NJ, :] = cd[b0s[None, :] + t_i[:, None], u_i[:, None], :]
    if LO:
        cdds4[0:6 * NL, NJ, :] = cd[B - NL:B].reshape(6 * NL, 4)
    cdds4 = np.ascontiguousarray(cdds4.reshape(RT, NJS * 4))
    bf = ml_dtypes.bfloat16
    clsw_pad = np.zeros((H, CP), np.float32)
    clsw_pad[:, 0:C] = cls_w
    clsw_pad = clsw_pad.astype(bf)
    in_maps = []
    for c in range(n_cores):
        rows = np.r_[c * DL:(c + 1) * DL, D + c * DL:D + (c + 1) * DL]
        in_maps.append({
            "pf": np.ascontiguousarray(
                part_feats[:, :, c * DL:(c + 1) * DL].reshape(B * NN, DL)),
            "cdds4": cdds4,
            "fcwT": np.ascontiguousarray(fc_w[rows, :].T).astype(bf),
            "clsw": clsw_pad,
            "biasr": bias,
            "consts": np.ascontiguousarray(
                np.concatenate([sel, maskc, ma, mp], axis=1)),
            "gsum": gs,
        })
    return in_maps


_NC_CACHE = {}


def kernel(part_feats, cdds, fc_w, fc_b, cls_w, cls_b):
    part_feats = np.ascontiguousarray(part_feats, dtype=np.float32)
    cdds = np.ascontiguousarray(cdds, dtype=np.float32)
    fc_w = np.ascontiguousarray(fc_w, dtype=np.float32)
    fc_b = np.ascontiguousarray(fc_b, dtype=np.float32)
    cls_w = np.ascontiguousarray(cls_w, dtype=np.float32)
    cls_b = np.ascontiguousarray(cls_b, dtype=np.float32)
    B = part_feats.shape[0]
    if "nc" not in _NC_CACHE:
        _NC_CACHE["nc"] = build_nc(n_cores=N_CORES)
    nc = _NC_CACHE["nc"]
    in_maps = make_host_inputs(part_feats, cdds, fc_w, fc_b, cls_w, cls_b,
                               n_cores=N_CORES)
    # First execution after compile has been observed to produce bad output
    # intermittently (runtime warmup); run once to warm up, then take the
    # second execution's result.
    bass_utils.run_bass_kernel_spmd(
        nc, in_maps, core_ids=list(range(N_CORES)))
    res = bass_utils.run_bass_kernel_spmd(
        nc, in_maps, core_ids=list(range(N_CORES)))
    # core c's "out" = samples [256c, 256(c+1))
    return np.concatenate([res.results[c]["out"] for c in range(N_CORES)],
                          axis=0)


# revision 6
# speedup vs baseline: 1.0400x; 1.0400x over previous
"""Trainium2 Bass kernel for nn_CNNtoGraph_77936476553433 (8-core k-parallel).

The GNN collapses algebraically: per sample b
    out[b] = x[b] @ W2 + bias,   x[b] = [(1/30) sum_u s[b,u] pf[b,u,:],
                                         (1/6)  sum_u        pf[b,u,:]]  (R^4096)
    W2 = fc_w @ cls_w  (4096x200),  bias = fc_b @ cls_w + cls_b
with s[b,u] = sum_v w[b,u,v] the edge-weight row sums from cdds box centers.

Sharding: the CONTRACTION dim k (=2*D) is split 8 ways. Each core streams the
full batch but only its 256-column d-slice of part_feats (12.6 MB), computes
s for all samples (stage 0, pipelined in 3 column-chunks), forms its 512-row
slice of xT (stage 1), computes its W2 k-slice from a host-pretransposed bf16
fc_w slice (no PE transposes, no AllGather), and accumulates partial outputs
out_part[b, c] over its k-slice (stage 2, sample-major PSUM so partials DMA
straight out). A 2-chunk bf16 ReduceScatter sums the 8 partials and hands
each core two 128-sample shards; bias is added once per shard afterwards.
"""
import sys
sys.path.insert(0, '/opt/trn_rl_repo')
import numpy as np
import ml_dtypes
import concourse.bass as bass
import concourse.bacc as bacc
import concourse.tile as tile
import concourse.mybir as mybir
from concourse import bass_utils

N_CORES = 8
B_FULL = 2048

F32 = mybir.dt.float32
F32R = mybir.dt.float32r
BF16 = mybir.dt.bfloat16
ALU = mybir.AluOpType
ACTF = mybir.ActivationFunctionType
ALPHA = 0.015

D, H, C, NN = 2048, 1024, 200, 6
CP = 256                    # C padded to a 512-byte bf16 line
RT, TS = 126, 21            # rows per sample-tile, samples per sample-tile
DL = D // N_CORES           # d-columns per core (256)
NKT = (2 * DL) // 128       # 4 k-tiles per core
NHT = H // 128              # 8 h-tiles
JC = 33                     # stage-0 pipeline chunk (NJS = 3*JC)


def ap_of(ap, offset, pattern):
    return bass.AP(ap.tensor, offset, pattern)


def ins_bcast(ap, idx, n):
    """Insert a broadcast (step-0) dim into an AP at position idx."""
    a = [list(d) for d in ap.ap]
    a.insert(idx, [0, n])
    return bass.AP(ap.tensor, ap.offset, a)


def bcast_last(ap, n):
    """Replace a singleton last dim with a step-0 broadcast of size n."""
    a = [list(d) for d in ap.ap]
    assert a[-1][1] == 1, a
    return bass.AP(ap.tensor, ap.offset, a[:-1] + [[0, n]])


def build_nc(B_loc=B_FULL, n_cores=8, **_):
    B = B_FULL                          # full batch on every core
    NJ = -(-B // TS)                    # 98 sample-tiles
    b0s = [TS * j for j in range(NJ - 1)] + [B - TS]
    LO = NJ * TS - B                    # overlap of last stile (10)
    NL = TS - LO                        # new samples in last stile (11)
    NJS = NJ + (1 if LO else 0)         # stage-0 columns (99)
    NBT = B // 128                      # 16 output b-tiles
    assert NJS == 3 * JC
    # pf super-tiles over tiles 0..NJ-2; tile NJ-1 (the overlap tail) is solo
    STJ = 7
    sts = []
    j0 = 0
    while j0 < NJ - 1:
        J = min(STJ, NJ - 1 - j0)
        sts.append((j0, J))
        j0 += J

    nc = bacc.Bacc("TRN2", target_bir_lowering=False, debug=False,
                   enable_asserts=True, num_devices=n_cores)
    pf = nc.dram_tensor("pf", [B * NN, DL], F32, kind="ExternalInput").ap()
    # box coords pre-gathered on host into stage-0's partition layout:
    # cdds4[p=(t,u), j, 0:4] = cdds[b0s[j] + t, 1+6*u : 5+6*u]
    cdds4 = nc.dram_tensor("cdds4", [RT, NJS * 4], F32,
                           kind="ExternalInput").ap()
    fcwT = nc.dram_tensor("fcwT", [H, 2 * DL], BF16, kind="ExternalInput").ap()
    clsw = nc.dram_tensor("clsw", [H, CP], BF16, kind="ExternalInput").ap()
    biasr = nc.dram_tensor("biasr", [1, C], F32, kind="ExternalInput").ap()
    # small f32 stage-0 constants in one tensor/DMA: [sel6 | mask_c | ma | mp]
    NCC = 6 + 6 + TS + TS
    consts = nc.dram_tensor("consts", [RT, NCC], F32, kind="ExternalInput").ap()
    gsum = nc.dram_tensor("gsum", [RT, RT], F32, kind="ExternalInput").ap()
    out = nc.dram_tensor("out", [2 * 128, C], F32, kind="ExternalOutput").ap()

    with tile.TileContext(nc) as tc:
        with tc.tile_pool(name="persist", bufs=1) as pp, \
             tc.tile_pool(name="dram", bufs=1, space="DRAM") as dp:

            # ---------------- persistent SBUF ----------------
            xT = pp.tile([128, NKT * B], BF16)           # stage-2 lhsT source
            wall = pp.tile([RT, NJS * 42], F32)          # stage-1 rhs (block diag)
            fcw_sb = pp.tile([128, NHT * 2 * DL], BF16)  # [h%128, ht, k]
            clsw_sb = pp.tile([128, NHT * CP], BF16)     # [h%128, ht, c]
            w2b = pp.tile([128, NKT * CP], BF16)         # [k%128, kt, c]
            bias_sb = pp.tile([1, C], F32)
            bias_bc = pp.tile([128, C], F32)
            ones_sb = pp.tile([1, 128], F32)
            scr_act = pp.tile([1, 8], F32)
            c_all = pp.tile([RT, NCC], F32)
            c_sel6 = c_all[:, 0:6]
            c_maskc = c_all[:, 6:12]
            c_ma21 = c_all[:, 12:12 + TS]
            c_mp21 = c_all[:, 12 + TS:NCC]
            # stage-0 working set
            own4 = pp.tile([RT, NJS * 4], F32)
            sxy = pp.tile([RT, NJS * 2], F32)
            rhs_all = pp.tile([RT, NJS * 12], F32)
            all_xy = pp.tile([RT, NJS * 12], F32)
            dall = pp.tile([RT, NJS * 6], F32)           # dx, d2, dist, em_minus
            dall2 = pp.tile([RT, NJS * 6], F32)          # dy, then relu scratch
            em = pp.tile([RT, NJS * 6], F32)
            esum = pp.tile([RT, NJS], F32)
            mean_sb = pp.tile([RT, NJS], F32)
            s_col = pp.tile([RT, NJS], F32)

            b_in = dp.tile([B, CP], BF16)
            b_out = dp.tile([256, CP], BF16)

            # -------- DMAs: consts + cdds4 + weights ahead of the pf stream
            c_gsum = pp.tile([RT, RT], F32)
            with tc.high_priority():
                nc.scalar.dma_start(c_all[:], consts)
                nc.scalar.dma_start(c_gsum[:], gsum)
                nc.scalar.dma_start(bias_sb[:], biasr)
                nc.sync.dma_start(own4[:], cdds4)
                nc.sync.dma_start(
                    fcw_sb[:].rearrange("p (ht k) -> p ht k", k=2 * DL),
                    fcwT.rearrange("(ht p) k -> p ht k", p=128))
                nc.sync.dma_start(
                    clsw_sb[:].rearrange("p (ht c) -> p ht c", c=CP),
                    clsw.rearrange("(ht p) c -> p ht c", p=128))
            # ACT table prewarm for Sqrt (Exp's load stays on the chain once)
            nc.gpsimd.memset(scr_act[:], 1.0)
            nc.scalar.activation(scr_act[:], scr_act[:], ACTF.Sqrt, scale=1.0)
            nc.gpsimd.memset(ones_sb[:], 1.0)
            # wall p-part is constant: wall[:, j, 21:42] = mp21 (all j)
            wv = wall[:].rearrange("p (j f) -> p j f", f=42)
            nc.gpsimd.tensor_copy(
                ap_of(wall[:], 21, [[NJS * 42, RT], [42, NJS], [1, TS]]),
                ins_bcast(c_mp21, 1, NJS))

            # ---------------- stage 0: edge weights (3 chunks) ------------
            o4 = own4[:].rearrange("p (j f) -> p j f", f=4)
            sx2 = sxy[:].rearrange("p (j f) -> p j f", f=2)
            r12 = rhs_all[:].rearrange("p (j f) -> p j f", f=12)
            a12 = all_xy[:].rearrange("p (j f) -> p j f", f=12)
            d6 = dall[:].rearrange("p (j f) -> p j f", f=6)
            e6 = dall2[:].rearrange("p (j f) -> p j f", f=6)
            m6 = em[:].rearrange("p (j f) -> p j f", f=6)
            CH = [slice(q * JC, (q + 1) * JC) for q in range(3)]

            def sel_(q):
                return ins_bcast(c_sel6, 1, JC)

            for q, s in enumerate(CH):
                nc.vector.tensor_add(sx2[:, s, 1:2], o4[:, s, 0:1],
                                     o4[:, s, 2:3])
                nc.vector.tensor_add(sx2[:, s, 0:1], o4[:, s, 1:2],
                                     o4[:, s, 3:4])
                nc.vector.tensor_mul(r12[:, s, 0:6], sel_(q),
                                     bcast_last(sx2[:, s, 0:1], 6))
                nc.vector.tensor_mul(r12[:, s, 6:12], sel_(q),
                                     bcast_last(sx2[:, s, 1:2], 6))

            with tc.tile_pool(name="ps0", bufs=1, space="PSUM") as ps0:
                for q, s in enumerate(CH):
                    gch = ps0.tile([RT, JC * 12], F32, tag="gch", bufs=2)
                    nc.tensor.matmul(gch[:], c_gsum[:],
                                     rhs_all[:, q * JC * 12:(q + 1) * JC * 12],
                                     start=True, stop=True)
                    nc.vector.tensor_copy(
                        all_xy[:, q * JC * 12:(q + 1) * JC * 12], gch[:])
                for q, s in enumerate(CH):
                    sx_b = bcast_last(sx2[:, s, 0:1], 6)
                    sy_b = bcast_last(sx2[:, s, 1:2], 6)
                    nc.gpsimd.tensor_sub(e6[:, s, :], sy_b, a12[:, s, 6:12])
                    nc.gpsimd.tensor_mul(e6[:, s, :], e6[:, s, :], e6[:, s, :])
                    nc.vector.tensor_sub(d6[:, s, :], sx_b, a12[:, s, 0:6])
                    nc.vector.tensor_mul(d6[:, s, :], d6[:, s, :], d6[:, s, :])
                    nc.vector.tensor_add(d6[:, s, :], d6[:, s, :], e6[:, s, :])
                # single sqrt + single exp (chunking would thrash the ACT
                # function table: 1.28us reload per Sqrt<->Exp switch)
                nc.scalar.activation(dall[:], dall[:], ACTF.Sqrt, scale=0.25)
                nc.scalar.activation(dall[:], dall[:], ACTF.Exp, scale=-ALPHA)
                for q, s in enumerate(CH):
                    # em = exp * mask ; esum = sum_v em
                    nc.vector.tensor_mul(m6[:, s, :], d6[:, s, :],
                                         ins_bcast(c_maskc, 1, JC))
                    nc.vector.tensor_reduce(
                        esum[:, s], m6[:, s, :], mybir.AxisListType.X,
                        ALU.add)
                for q, s in enumerate(CH):
                    mps = ps0.tile([RT, JC], F32, tag="mps", bufs=2)
                    nc.tensor.matmul(mps[:], c_gsum[:], esum[:, s],
                                     start=True, stop=True)
                    nc.vector.tensor_copy(mean_sb[:, s], mps[:])
                for q, s in enumerate(CH):
                    # em_minus = em - mean/30 ; s' = sum_v relu(em_minus)
                    # (the 0.8 = SCALE/30 factor is folded into ma21)
                    nc.vector.scalar_tensor_tensor(
                        d6[:, s, :], ins_bcast(mean_sb[:, s], 2, 6),
                        -1.0 / 30.0, m6[:, s, :], op0=ALU.mult, op1=ALU.add)
                    nc.vector.tensor_relu(e6[:, s, :], d6[:, s, :])
                    nc.vector.tensor_reduce(s_col[:, s], e6[:, s, :],
                                            mybir.AxisListType.X, ALU.add)
                    # wall a-part: wall[:, j, 0:21] = (0.8*ma21) * s'[:, j]
                    nc.vector.tensor_mul(
                        ap_of(wall[:], q * JC * 42,
                              [[NJS * 42, RT], [42, JC], [1, TS]]),
                        ins_bcast(c_ma21, 1, JC),
                        ins_bcast(s_col[:, s], 2, TS))

            # ---------------- W2 k-slice + bias broadcast ----------------
            fS = fcw_sb[:].rearrange("p (ht k) -> p ht k", k=2 * DL)
            cS = clsw_sb[:].rearrange("p (ht c) -> p ht c", c=CP)
            w2v = w2b[:].rearrange("p (kt c) -> p kt c", c=CP)
            with tc.tile_pool(name="psw", bufs=1, space="PSUM") as psw:
                for kt in range(NKT):
                    wps = psw.tile([128, CP], F32, tag="wps", bufs=2)
                    for ht in range(NHT):
                        nc.tensor.matmul(
                            wps[:], fS[:, ht, kt * 128:(kt + 1) * 128],
                            cS[:, ht, :],
                            start=(ht == 0), stop=(ht == NHT - 1))
                    nc.vector.tensor_copy(w2v[:, kt, :], wps[:])
                bps = psw.tile([128, C], F32, tag="bps")
                nc.tensor.matmul(bps[:], ones_sb[:], bias_sb[:], start=True,
                                 stop=True)
                nc.vector.tensor_copy(bias_bc[:], bps[:])

            # ---------------- stage 1 + interleaved stage 2 ----------------
            xv = xT[:].rearrange("p (kt b) -> p kt b", b=B)
            lo, nl = LO, NL
            state = {"bt": 0, "ev": 0, "pt": None}
            with tc.tile_pool(name="pfp", bufs=7) as pfp, \
                 tc.tile_pool(name="ps1", bufs=1, space="PSUM") as ps1, \
                 tc.tile_pool(name="ps2", bufs=1, space="PSUM") as ps2, \
                 tc.tile_pool(name="ptp", bufs=8) as ptp:

                def do_btile(t):
                    ops = ps2.tile([128, CP], F32, tag="ops", bufs=2)
                    for kt in range(NKT):
                        nc.tensor.matmul(
                            ops[:],
                            ap_of(xT[:], kt * B + 128 * t,
                                  [[NKT * B, 128], [1, 128]]),
                            w2v[:, kt, :],
                            start=(kt == 0), stop=(kt == NKT - 1))
                    if t % 2 == 0:
                        state["pt"] = ptp.tile([128, 2 * CP], BF16, tag="pt",
                                               bufs=4, name="ptb")
                    pt = state["pt"]
                    nc.scalar.copy(pt[:, (t % 2) * CP:(t % 2 + 1) * CP], ops[:])
                    if t % 2 == 1:
                        # rows 128(t-1) .. 128(t+1) of b_in in one DMA
                        nc.gpsimd.dma_start(
                            ap_of(b_in.opt(), (t - 1) * 128 * CP,
                                  [[CP, 128], [128 * CP, 2], [1, CP]]),
                            ap_of(pt[:], 0, [[2 * CP, 128], [CP, 2], [1, CP]]))
                    if t == NBT - 1:
                        nc.gpsimd.collective_compute(
                            "ReduceScatter", ALU.add,
                            replica_groups=[list(range(n_cores))],
                            ins=[b_in.opt()], outs=[b_out.opt()])

                def post():
                    # shard rows (h*128+p) -> SBUF [p, h, c]; add bias; store
                    pb = pp.tile([128, 2 * CP], BF16, name="pb")
                    pfq = pp.tile([128, 2 * C], F32, name="pfq")
                    nc.sync.dma_start(
                        pb[:].rearrange("p (h c) -> p h c", c=CP),
                        b_out.opt().rearrange("(h p) c -> p h c", p=128))
                    nc.vector.tensor_add(
                        pfq[:].rearrange("p (h c) -> p h c", c=C),
                        ap_of(pb[:], 0, [[2 * CP, 128], [CP, 2], [1, C]]),
                        ins_bcast(bias_bc[:], 1, 2))
                    nc.sync.dma_start(
                        out.rearrange("(h p) c -> p h c", p=128),
                        pfq[:].rearrange("p (h c) -> p h c", c=C))

                def do_tile(j, pft_ap):
                    last = j == NJ - 1 and lo > 0
                    rt = 6 * nl if last else RT
                    ns = nl if last else TS
                    if last:
                        rhs_w = ap_of(wall[:], (NJS - 1) * 42,
                                      [[NJS * 42, rt], [21, 2], [1, ns]])
                    else:
                        rhs_w = wv[:, j, :]
                    psA = ps1.tile([128, 84], F32, tag="psA", bufs=4)
                    for db in range(2):
                        nc.tensor.matmul(
                            psA[:, db * 2 * ns:(db + 1) * 2 * ns],
                            pft_ap[0:rt, db * 128:(db + 1) * 128],
                            rhs_w, start=True, stop=True)
                    c0 = b0s[j] + lo if last else b0s[j]
                    # one 4D copy: psA[p, db, h, s] -> xT[p, (2h+db)*B + c0+s]
                    src = ap_of(psA[:], 0,
                                [[84, 128], [ns, 2], [2 * ns, 2], [1, ns]])
                    dst = ap_of(xT[:], c0,
                                [[NKT * B, 128], [2 * B, 2], [B, 2], [1, ns]])
                    if state["ev"] % 2 == 0:
                        nc.vector.tensor_copy(dst, src)
                    else:
                        nc.scalar.copy(dst, src)
                    state["ev"] += 1
                    if last:
                        return   # tail runs first; it must not advance bt
                    cov = TS * (j + 1)   # samples covered by regular tiles
                    while state["bt"] < NBT and (
                            128 * (state["bt"] + 1) <= cov
                            or (state["bt"] == NBT - 1 and cov >= B - NL)):
                        do_btile(state["bt"])
                        state["bt"] += 1

                pf_tl = pfp.tile([6 * NL, DL], F32, tag="pftail", bufs=1)
                nc.sync.dma_start(pf_tl[:], pf[(B - NL) * 6:B * 6, :])
                do_tile(NJ - 1, pf_tl[:])
                for si, (j0, J) in enumerate(sts):
                    pf_st = pfp.tile([RT, STJ * DL], F32, tag="pf", bufs=7)
                    nc.sync.dma_start(
                        pf_st[:, 0:J * DL].rearrange(
                            "p (jj d) -> p jj d", d=DL),
                        ap_of(pf, j0 * RT * DL,
                              [[DL, RT], [RT * DL, J], [1, DL]]))
                    for jj in range(J):
                        do_tile(j0 + jj, pf_st[:, jj * DL:(jj + 1) * DL])
                while state["bt"] < NBT:
                    do_btile(state["bt"])
                    state["bt"] += 1
                post()
    nc.compile()
    return nc


def make_host_inputs(part_feats, cdds, fc_w, fc_b, cls_w, cls_b, n_cores=8):
    """Shard + prepare per-core in_maps from full inputs."""
    B = part_feats.shape[0]
    p = np.arange(RT)
    maskc = (p[:, None] % 6 != np.arange(6)[None, :]).astype(np.float32)
    sel = (p[:, None] % 6 == np.arange(6)[None, :]).astype(np.float32)
    gs = (p[:, None] // 6 == p[None, :] // 6).astype(np.float32)
    ma = np.zeros((RT, TS), np.float32)
    ma[p, p // 6] = 0.8                  # SCALE/30 folded in
    mp = np.zeros((RT, TS), np.float32)
    mp[p, p // 6] = 1.0 / 6.0
    bias = (fc_b @ cls_w + cls_b).reshape(1, C).astype(np.float32)
    # cdds4[p=(t,u), j, 0:4] = cdds[b0s[j] + t, 1+6u : 5+6u]; tail column
    # (j = NJ) = the last NL samples at partitions 0:6*NL, zero-padded.
    NJ = -(-B // TS)
    LO = NJ * TS - B
    NL = TS - LO
    NJS = NJ + (1 if LO else 0)
    b0s = np.array([TS * j for j in range(NJ - 1)] + [B - TS])
    cd = cdds.reshape(B, 6, 6)[:, :, 1:5]                # (B, u, 4)
    t_i, u_i = p // 6, p % 6
    cdds4 = np.zeros((RT, NJS, 4), np.float32)
    cdds4[:, 0:NJ, :] = cd[b0s[None, :] + t_i[:, None], u_i[:, None], :]
    if LO:
        cdds4[0:6 * NL, NJ, :] = cd[B - NL:B].reshape(6 * NL, 4)
    cdds4 = np.ascontiguousarray(cdds4.reshape(RT, NJS * 4))
    bf = ml_dtypes.bfloat16
    clsw_pad = np.zeros((H, CP), np.float32)
    clsw_pad[:, 0:C] = cls_w
    clsw_pad = clsw_pad.astype(bf)
    in_maps = []
    for c in range(n_cores):
        rows = np.r_[c * DL:(c + 1) * DL, D + c * DL:D + (c + 1) * DL]
        in_maps.append({
            "pf": np.ascontiguousarray(
                part_feats[:, :, c * DL:(c + 1) * DL].reshape(B * NN, DL)),
            "cdds4": cdds4,
            "fcwT": np.ascontiguousarray(fc_w[rows, :].T).astype(bf),
            "clsw": clsw_pad,
            "biasr": bias,
            "consts": np.ascontiguousarray(
                np.concatenate([sel, maskc, ma, mp], axis=1)),
            "gsum": gs,
        })
    return in_maps


_NC_CACHE = {}


def kernel(part_feats, cdds, fc_w, fc_b, cls_w, cls_b):
    part_feats = np.ascontiguousarray(part_feats, dtype=np.float32)
    cdds = np.ascontiguousarray(cdds, dtype=np.float32)
    fc_w = np.ascontiguousarray(fc_w, dtype=np.float32)
    fc_b = np.ascontiguousarray(fc_b, dtype=np.float32)
    cls_w = np.ascontiguousarray(cls_w, dtype=np.float32)
    cls_b = np.ascontiguousarray(cls_b, dtype=np.float32)
    B = part_feats.shape[0]
    if "nc" not in _NC_CACHE:
        _NC_CACHE["nc"] = build_nc(n_cores=N_CORES)
    nc = _NC_CACHE["nc"]
    in_maps = make_host_inputs(part_feats, cdds, fc_w, fc_b, cls_w, cls_b,
                               n_cores=N_CORES)
    # First execution after compile has been observed to produce bad output
    # intermittently (runtime warmup); run once to warm up, then take the
    # second execution's result.
    bass_utils.run_bass_kernel_spmd(
        nc, in_maps, core_ids=list(range(N_CORES)))
    res = bass_utils.run_bass_kernel_spmd(
        nc, in_maps, core_ids=list(range(N_CORES)))
    # core c's "out" = samples [256c, 256(c+1))
    return np.concatenate([res.results[c]["out"] for c in range(N_CORES)],
                          axis=0)


# revision 13
# speedup vs baseline: 1.0807x; 1.0391x over previous
"""Trainium2 Bass kernel for nn_CNNtoGraph_77936476553433 (8-core k-parallel).

The GNN collapses algebraically: per sample b
    out[b] = x[b] @ W2 + bias,   x[b] = [(1/30) sum_u s[b,u] pf[b,u,:],
                                         (1/6)  sum_u        pf[b,u,:]]  (R^4096)
    W2 = fc_w @ cls_w  (4096x200),  bias = fc_b @ cls_w + cls_b
with s[b,u] = sum_v w[b,u,v] the edge-weight row sums from cdds box centers.

Sharding: the CONTRACTION dim k (=2*D) is split 8 ways. Each core streams the
full batch but only its 256-column d-slice of part_feats (12.6 MB), computes
s for all samples (stage 0, pipelined in 3 column-chunks), forms its 512-row
slice of xT (stage 1), computes its W2 k-slice from a host-pretransposed bf16
fc_w slice (no PE transposes, no AllGather), and accumulates partial outputs
out_part[b, c] over its k-slice (stage 2, sample-major PSUM so partials DMA
straight out). One bf16 ReduceScatter sums the 8 partials and hands each
core its contiguous 256-sample shard; bias is added once after reduction.
"""
import sys
sys.path.insert(0, '/opt/trn_rl_repo')
import numpy as np
import ml_dtypes
import concourse.bass as bass
import concourse.bacc as bacc
import concourse.tile as tile
import concourse.mybir as mybir
from concourse import bass_utils

N_CORES = 8
B_FULL = 2048

F32 = mybir.dt.float32
F32R = mybir.dt.float32r
BF16 = mybir.dt.bfloat16
FP8 = mybir.dt.float8e4
ALU = mybir.AluOpType
ACTF = mybir.ActivationFunctionType
ALPHA = 0.015

D, H, C, NN = 2048, 1024, 200, 6
CP = 256                    # C padded to a 512-byte bf16 line
RT, TS = 126, 21            # rows per sample-tile, samples per sample-tile
DL = D // N_CORES           # d-columns per core (256)
NKT = (2 * DL) // 128       # 4 k-tiles per core
NHT = H // 128              # 8 h-tiles
JC = 33                     # stage-0 pipeline chunk (NJS = 3*JC)


def ap_of(ap, offset, pattern):
    return bass.AP(ap.tensor, offset, pattern)


def ins_bcast(ap, idx, n):
    """Insert a broadcast (step-0) dim into an AP at position idx."""
    a = [list(d) for d in ap.ap]
    a.insert(idx, [0, n])
    return bass.AP(ap.tensor, ap.offset, a)


def bcast_last(ap, n):
    """Replace a singleton last dim with a step-0 broadcast of size n."""
    a = [list(d) for d in ap.ap]
    assert a[-1][1] == 1, a
    return bass.AP(ap.tensor, ap.offset, a[:-1] + [[0, n]])


def build_nc(B_loc=B_FULL, n_cores=8, **_):
    B = B_FULL                          # full batch on every core
    NJ = -(-B // TS)                    # 98 sample-tiles
    b0s = [TS * j for j in range(NJ - 1)] + [B - TS]
    LO = NJ * TS - B                    # overlap of last stile (10)
    NL = TS - LO                        # new samples in last stile (11)
    NJS = NJ + (1 if LO else 0)         # stage-0 columns (99)
    NBT = B // 128                      # 16 output b-tiles
    assert NJS == 3 * JC
    # pf super-tiles over tiles 0..NJ-2; tile NJ-1 (the overlap tail) is solo
    STJ = 7
    sts = []
    j0 = 0
    while j0 < NJ - 1 - 7:
        J = min(STJ, NJ - 1 - 7 - j0)
        sts.append((j0, J))
        j0 += J
    for J in (4, 2, 1):
        sts.append((j0, J))
        j0 += J
    assert j0 == NJ - 1

    nc = bacc.Bacc("TRN2", target_bir_lowering=False, debug=False,
                   enable_asserts=True, num_devices=n_cores)
    pf = nc.dram_tensor("pf", [B * NN, DL], F32, kind="ExternalInput").ap()
    # box coords pre-gathered on host into stage-0's partition layout:
    # cdds4[p=(t,u), j, 0:4] = cdds[b0s[j] + t, 1+6*u : 5+6*u]
    cdds4 = nc.dram_tensor("cdds4", [RT, NJS * 4], F32,
                           kind="ExternalInput").ap()
    fcwT = nc.dram_tensor("fcwT", [H, 2 * DL], BF16, kind="ExternalInput").ap()
    clsw = nc.dram_tensor("clsw", [H, CP], BF16, kind="ExternalInput").ap()
    biasr = nc.dram_tensor("biasr", [1, C], F32, kind="ExternalInput").ap()
    # small f32 stage-0 constants in one tensor/DMA: [sel6 | mask_c | ma | mp]
    NCC = 6 + 6 + TS + TS
    consts = nc.dram_tensor("consts", [RT, NCC], F32, kind="ExternalInput").ap()
    gsum = nc.dram_tensor("gsum", [RT, RT], F32, kind="ExternalInput").ap()
    out = nc.dram_tensor("out", [2 * 128, C], F32, kind="ExternalOutput").ap()

    with tile.TileContext(nc) as tc:
        with tc.tile_pool(name="persist", bufs=1) as pp, \
             tc.tile_pool(name="dram", bufs=1, space="DRAM") as dp:

            # ---------------- persistent SBUF ----------------
            xT = pp.tile([128, NKT * B], BF16)           # stage-2 lhsT source
            wall = pp.tile([RT, NJS * 42], F32)          # stage-1 rhs (block diag)
            fcw_sb = pp.tile([128, NHT * 2 * DL], BF16)  # [h%128, ht, k]
            clsw_sb = pp.tile([128, NHT * CP], BF16)     # [h%128, ht, c]
            w2b = pp.tile([128, NKT * CP], BF16)         # [k%128, kt, c]
            bias_sb = pp.tile([1, C], F32)
            bias_bc = pp.tile([128, C], F32)
            ones_sb = pp.tile([1, 128], F32)
            scr_act = pp.tile([1, 8], F32)
            c_all = pp.tile([RT, NCC], F32)
            c_sel6 = c_all[:, 0:6]
            c_maskc = c_all[:, 6:12]
            c_ma21 = c_all[:, 12:12 + TS]
            c_mp21 = c_all[:, 12 + TS:NCC]
            # stage-0 working set
            own4 = pp.tile([RT, NJS * 4], F32)
            sxy = pp.tile([RT, NJS * 2], F32)
            rhs_all = pp.tile([RT, NJS * 12], F32)
            all_xy = pp.tile([RT, NJS * 12], F32)
            dall = pp.tile([RT, NJS * 6], F32)           # dx, d2, dist, em_minus
            dall2 = pp.tile([RT, NJS * 6], F32)          # dy, then relu scratch
            em = pp.tile([RT, NJS * 6], F32)
            esum = pp.tile([RT, NJS], F32)
            mean_sb = pp.tile([RT, NJS], F32)
            s_col = pp.tile([RT, NJS], F32)

            b_in = dp.tile([B, CP], BF16)
            b_out = dp.tile([256, CP], BF16)

            # -------- DMAs: consts + cdds4 + weights ahead of the pf stream
            c_gsum = pp.tile([RT, RT], F32)
            with tc.high_priority():
                nc.sync.dma_start(own4[:], cdds4)
                nc.sync.dma_start(c_gsum[:], gsum)
                nc.scalar.dma_start(c_all[:], consts)
                nc.scalar.dma_start(bias_sb[:], biasr)
                nc.sync.dma_start(
                    fcw_sb[:].rearrange("p (ht k) -> p ht k", k=2 * DL),
                    fcwT.rearrange("(ht p) k -> p ht k", p=128))
                nc.sync.dma_start(
                    clsw_sb[:].rearrange("p (ht c) -> p ht c", c=CP),
                    clsw.rearrange("(ht p) c -> p ht c", p=128))
            # ACT table prewarm for Sqrt (Exp's load stays on the chain once)
            nc.gpsimd.memset(scr_act[:], 1.0)
            nc.scalar.activation(scr_act[:], scr_act[:], ACTF.Sqrt, scale=1.0)
            nc.gpsimd.memset(ones_sb[:], 1.0)
            # wall p-part is constant: wall[:, j, 21:42] = mp21 (all j)
            wv = wall[:].rearrange("p (j f) -> p j f", f=42)
            nc.gpsimd.tensor_copy(
                ap_of(wall[:], 21, [[NJS * 42, RT], [42, NJS], [1, TS]]),
                ins_bcast(c_mp21, 1, NJS))

            # ---------------- stage 0: edge weights (3 chunks) ------------
            o4 = own4[:].rearrange("p (j f) -> p j f", f=4)
            sx2 = sxy[:].rearrange("p (j f) -> p j f", f=2)
            r12 = rhs_all[:].rearrange("p (j f) -> p j f", f=12)
            a12 = all_xy[:].rearrange("p (j f) -> p j f", f=12)
            d6 = dall[:].rearrange("p (j f) -> p j f", f=6)
            e6 = dall2[:].rearrange("p (j f) -> p j f", f=6)
            m6 = em[:].rearrange("p (j f) -> p j f", f=6)
            CH = [slice(q * JC, (q + 1) * JC) for q in range(3)]

            def sel_(q):
                return ins_bcast(c_sel6, 1, JC)

            for q, s in enumerate(CH):
                nc.vector.tensor_add(sx2[:, s, 1:2], o4[:, s, 0:1],
                                     o4[:, s, 2:3])
                nc.vector.tensor_add(sx2[:, s, 0:1], o4[:, s, 1:2],
                                     o4[:, s, 3:4])
                nc.vector.tensor_mul(r12[:, s, 0:6], sel_(q),
                                     bcast_last(sx2[:, s, 0:1], 6))
                nc.vector.tensor_mul(r12[:, s, 6:12], sel_(q),
                                     bcast_last(sx2[:, s, 1:2], 6))

            with tc.tile_pool(name="ps0", bufs=1, space="PSUM") as ps0:
                for q, s in enumerate(CH):
                    gch = ps0.tile([RT, JC * 12], F32, tag="gch", bufs=2)
                    nc.tensor.matmul(gch[:], c_gsum[:],
                                     rhs_all[:, q * JC * 12:(q + 1) * JC * 12],
                                     start=True, stop=True)
                    nc.vector.tensor_copy(
                        all_xy[:, q * JC * 12:(q + 1) * JC * 12], gch[:])
                for q, s in enumerate(CH):
                    sx_b = bcast_last(sx2[:, s, 0:1], 6)
                    sy_b = bcast_last(sx2[:, s, 1:2], 6)
                    nc.gpsimd.tensor_sub(e6[:, s, :], sy_b, a12[:, s, 6:12])
                    nc.gpsimd.tensor_mul(e6[:, s, :], e6[:, s, :], e6[:, s, :])
                    nc.vector.tensor_sub(d6[:, s, :], sx_b, a12[:, s, 0:6])
                    nc.vector.tensor_mul(d6[:, s, :], d6[:, s, :], d6[:, s, :])
                    nc.vector.tensor_add(d6[:, s, :], d6[:, s, :], e6[:, s, :])
                # single sqrt + single exp (chunking would thrash the ACT
                # function table: 1.28us reload per Sqrt<->Exp switch)
                nc.scalar.activation(dall[:], dall[:], ACTF.Sqrt, scale=0.25)
                nc.scalar.activation(dall[:], dall[:], ACTF.Exp, scale=-ALPHA)
                for q, s in enumerate(CH):
                    # em = exp * mask ; esum = sum_v em
                    nc.vector.tensor_mul(m6[:, s, :], d6[:, s, :],
                                         ins_bcast(c_maskc, 1, JC))
                    nc.vector.tensor_reduce(
                        esum[:, s], m6[:, s, :], mybir.AxisListType.X,
                        ALU.add)
                for q, s in enumerate(CH):
                    mps = ps0.tile([RT, JC], F32, tag="mps", bufs=2)
                    nc.tensor.matmul(mps[:], c_gsum[:], esum[:, s],
                                     start=True, stop=True)
                    nc.vector.tensor_copy(mean_sb[:, s], mps[:])
                for q, s in enumerate(CH):
                    # em_minus = em - mean/30 ; s' = sum_v relu(em_minus)
                    # (the 0.8 = SCALE/30 factor is folded into ma21)
                    nc.vector.scalar_tensor_tensor(
                        d6[:, s, :], ins_bcast(mean_sb[:, s], 2, 6),
                        -1.0 / 30.0, m6[:, s, :], op0=ALU.mult, op1=ALU.add)
                    nc.vector.tensor_relu(e6[:, s, :], d6[:, s, :])
                    nc.vector.tensor_reduce(s_col[:, s], e6[:, s, :],
                                            mybir.AxisListType.X, ALU.add)
                    # wall a-part: wall[:, j, 0:21] = (0.8*ma21) * s'[:, j]
                    nc.vector.tensor_mul(
                        ap_of(wall[:], q * JC * 42,
                              [[NJS * 42, RT], [42, JC], [1, TS]]),
                        ins_bcast(c_ma21, 1, JC),
                        ins_bcast(s_col[:, s], 2, TS))

            # ---------------- W2 k-slice + bias broadcast ----------------
            fS = fcw_sb[:].rearrange("p (ht k) -> p ht k", k=2 * DL)
            cS = clsw_sb[:].rearrange("p (ht c) -> p ht c", c=CP)
            w2v = w2b[:].rearrange("p (kt c) -> p kt c", c=CP)
            with tc.tile_pool(name="psw", bufs=1, space="PSUM") as psw:
                for kt in range(NKT):
                    wps = psw.tile([128, CP], F32, tag="wps", bufs=2)
                    for ht in range(NHT):
                        nc.tensor.matmul(
                            wps[:], fS[:, ht, kt * 128:(kt + 1) * 128],
                            cS[:, ht, :],
                            start=(ht == 0), stop=(ht == NHT - 1))
                    nc.vector.tensor_copy(w2v[:, kt, :], wps[:])
                bps = psw.tile([128, C], F32, tag="bps")
                nc.tensor.matmul(bps[:], ones_sb[:], bias_sb[:], start=True,
                                 stop=True)
                nc.vector.tensor_copy(bias_bc[:], bps[:])

            # ---------------- stage 1 + interleaved stage 2 ----------------
            xv = xT[:].rearrange("p (kt b) -> p kt b", b=B)
            lo, nl = LO, NL
            state = {"bt": 0, "ev": 0, "pt": None}
            with tc.tile_pool(name="pfp", bufs=7) as pfp, \
                 tc.tile_pool(name="ps1", bufs=1, space="PSUM") as ps1, \
                 tc.tile_pool(name="ps2", bufs=1, space="PSUM") as ps2, \
                 tc.tile_pool(name="ptp", bufs=8) as ptp:

                def do_btile(t):
                    ops = ps2.tile([128, CP], F32, tag="ops", bufs=2)
                    for kt in range(NKT):
                        nc.tensor.matmul(
                            ops[:],
                            ap_of(xT[:], kt * B + 128 * t,
                                  [[NKT * B, 128], [1, 128]]),
                            w2v[:, kt, :],
                            start=(kt == 0), stop=(kt == NKT - 1))
                    solo = t >= NBT - 2
                    if t % 2 == 0 and not solo:
                        state["pt"] = ptp.tile([128, 2 * CP], BF16, tag="pt",
                                               bufs=4, name="ptb")
                    if solo:
                        pts = ptp.tile([128, CP], BF16, tag="pts", bufs=4,
                                       name="pts")
                        nc.scalar.copy(pts[:], ops[:])
                        nc.gpsimd.dma_start(
                            ap_of(b_in.opt(), t * 128 * CP,
                                  [[CP, 128], [1, CP]]),
                            pts[:])
                    else:
                        pt = state["pt"]
                        nc.scalar.copy(pt[:, (t % 2) * CP:(t % 2 + 1) * CP],
                                       ops[:])
                        if t % 2 == 1:
                            # rows 128(t-1) .. 128(t+1) of b_in in one DMA
                            nc.gpsimd.dma_start(
                                ap_of(b_in.opt(), (t - 1) * 128 * CP,
                                      [[CP, 128], [128 * CP, 2], [1, CP]]),
                                ap_of(pt[:], 0,
                                      [[2 * CP, 128], [CP, 2], [1, CP]]))
                    if t == NBT - 1:
                        nc.gpsimd.collective_compute(
                            "ReduceScatter", ALU.add,
                            replica_groups=[list(range(n_cores))],
                            ins=[b_in.opt()], outs=[b_out.opt()])

                def post():
                    # shard rows (h*128+p) -> SBUF [p, h, c]; add bias; store
                    pb = pp.tile([128, 2 * CP], BF16, name="pb")
                    pfq = pp.tile([128, 2 * C], F32, name="pfq")
                    nc.sync.dma_start(
                        pb[:].rearrange("p (h c) -> p h c", c=CP),
                        b_out.opt().rearrange("(h p) c -> p h c", p=128))
                    nc.vector.tensor_add(
                        pfq[:].rearrange("p (h c) -> p h c", c=C),
                        ap_of(pb[:], 0, [[2 * CP, 128], [CP, 2], [1, C]]),
                        ins_bcast(bias_bc[:], 1, 2))
                    nc.sync.dma_start(
                        out.rearrange("(h p) c -> p h c", p=128),
                        pfq[:].rearrange("p (h c) -> p h c", c=C))

                def do_tile(j, pft_ap):
                    last = j == NJ - 1 and lo > 0
                    rt = 6 * nl if last else RT
                    ns = nl if last else TS
                    if last:
                        rhs_w = ap_of(wall[:], (NJS - 1) * 42,
                                      [[NJS * 42, rt], [21, 2], [1, ns]])
                    else:
                        rhs_w = wv[:, j, :]
                    psA = ps1.tile([128, 84], F32, tag="psA", bufs=4)
                    for db in range(2):
                        nc.tensor.matmul(
                            psA[:, db * 2 * ns:(db + 1) * 2 * ns],
                            pft_ap[0:rt, db * 128:(db + 1) * 128],
                            rhs_w, start=True, stop=True)
                    c0 = b0s[j] + lo if last else b0s[j]
                    # one 4D copy: psA[p, db, h, s] -> xT[p, (2h+db)*B + c0+s]
                    src = ap_of(psA[:], 0,
                                [[84, 128], [ns, 2], [2 * ns, 2], [1, ns]])
                    dst = ap_of(xT[:], c0,
                                [[NKT * B, 128], [2 * B, 2], [B, 2], [1, ns]])
                    if state["ev"] % 2 == 0:
                        nc.vector.tensor_copy(dst, src)
                    else:
                        nc.scalar.copy(dst, src)
                    state["ev"] += 1
                    if last:
                        return   # tail runs first; it must not advance bt
                    cov = TS * (j + 1)   # samples covered by regular tiles
                    while state["bt"] < NBT and (
                            128 * (state["bt"] + 1) <= cov
                            or (state["bt"] == NBT - 1 and cov >= B - NL)):
                        do_btile(state["bt"])
                        state["bt"] += 1

                pf_tl = pfp.tile([6 * NL, DL], F32, tag="pftail", bufs=1)
                nc.sync.dma_start(pf_tl[:], pf[(B - NL) * 6:B * 6, :])
                do_tile(NJ - 1, pf_tl[:])
                for si, (j0, J) in enumerate(sts):
                    pf_st = pfp.tile([RT, STJ * DL], F32, tag="pf", bufs=7)
                    nc.sync.dma_start(
                        pf_st[:, 0:J * DL].rearrange(
                            "p (jj d) -> p jj d", d=DL),
                        ap_of(pf, j0 * RT * DL,
                              [[DL, RT], [RT * DL, J], [1, DL]]))
                    for jj in range(J):
                        do_tile(j0 + jj, pf_st[:, jj * DL:(jj + 1) * DL])
                while state["bt"] < NBT:
                    do_btile(state["bt"])
                    state["bt"] += 1
                post()
    nc.compile()
    return nc


def make_host_inputs(part_feats, cdds, fc_w, fc_b, cls_w, cls_b, n_cores=8):
    """Shard + prepare per-core in_maps from full inputs."""
    B = part_feats.shape[0]
    p = np.arange(RT)
    maskc = (p[:, None] % 6 != np.arange(6)[None, :]).astype(np.float32)
    sel = (p[:, None] % 6 == np.arange(6)[None, :]).astype(np.float32)
    gs = (p[:, None] // 6 == p[None, :] // 6).astype(np.float32)
    ma = np.zeros((RT, TS), np.float32)
    ma[p, p // 6] = 0.8                  # SCALE/30 folded in
    mp = np.zeros((RT, TS), np.float32)
    mp[p, p // 6] = 1.0 / 6.0
    bias = (fc_b @ cls_w + cls_b).reshape(1, C).astype(np.float32)
    # cdds4[p=(t,u), j, 0:4] = cdds[b0s[j] + t, 1+6u : 5+6u]; tail column
    # (j = NJ) = the last NL samples at partitions 0:6*NL, zero-padded.
    NJ = -(-B // TS)
    LO = NJ * TS - B
    NL = TS - LO
    NJS = NJ + (1 if LO else 0)
    b0s = np.array([TS * j for j in range(NJ - 1)] + [B - TS])
    cd = cdds.reshape(B, 6, 6)[:, :, 1:5]                # (B, u, 4)
    t_i, u_i = p // 6, p % 6
    cdds4 = np.zeros((RT, NJS, 4), np.float32)
    cdds4[:, 0:NJ, :] = cd[b0s[None, :] + t_i[:, None], u_i[:, None], :]
    if LO:
        cdds4[0:6 * NL, NJ, :] = cd[B - NL:B].reshape(6 * NL, 4)
    cdds4 = np.ascontiguousarray(cdds4.reshape(RT, NJS * 4))
    bf = ml_dtypes.bfloat16
    f8 = ml_dtypes.float8_e4m3fn
    clsw_pad = np.zeros((H, CP), np.float32)
    clsw_pad[:, 0:C] = cls_w
    clsw_pad = clsw_pad.astype(bf)
    in_maps = []
    for c in range(n_cores):
        rows = np.r_[c * DL:(c + 1) * DL, D + c * DL:D + (c + 1) * DL]
        in_maps.append({
            "pf": np.ascontiguousarray(
                part_feats[:, :, c * DL:(c + 1) * DL].reshape(B * NN, DL)),
            "cdds4": cdds4,
            "fcwT": np.ascontiguousarray(fc_w[rows, :].T).astype(bf),
            "clsw": clsw_pad,
            "biasr": bias,
            "consts": np.ascontiguousarray(
                np.concatenate([sel, maskc, ma, mp], axis=1)),
            "gsum": gs,
        })
    return in_maps


_NC_CACHE = {}


def kernel(part_feats, cdds, fc_w, fc_b, cls_w, cls_b):
    part_feats = np.ascontiguousarray(part_feats, dtype=np.float32)
    cdds = np.ascontiguousarray(cdds, dtype=np.float32)
    fc_w = np.ascontiguousarray(fc_w, dtype=np.float32)
    fc_b = np.ascontiguousarray(fc_b, dtype=np.float32)
    cls_w = np.ascontiguousarray(cls_w, dtype=np.float32)
    cls_b = np.ascontiguousarray(cls_b, dtype=np.float32)
    B = part_feats.shape[0]
    if "nc" not in _NC_CACHE:
        _NC_CACHE["nc"] = build_nc(n_cores=N_CORES)
    nc = _NC_CACHE["nc"]
    in_maps = make_host_inputs(part_feats, cdds, fc_w, fc_b, cls_w, cls_b,
                               n_cores=N_CORES)
    # First execution after compile has been observed to produce bad output
    # intermittently (runtime warmup); run once to warm up, then take the
    # second execution's result.
    bass_utils.run_bass_kernel_spmd(
        nc, in_maps, core_ids=list(range(N_CORES)))
    res = bass_utils.run_bass_kernel_spmd(
        nc, in_maps, core_ids=list(range(N_CORES)))
    # core c's "out" = samples [256c, 256(c+1))
    return np.concatenate([res.results[c]["out"] for c in range(N_CORES)],
                          axis=0)


# revision 19
# speedup vs baseline: 1.0923x; 1.0108x over previous
"""Trainium2 Bass kernel for nn_CNNtoGraph_77936476553433 (8-core k-parallel).

The GNN collapses algebraically: per sample b
    out[b] = x[b] @ W2 + bias,   x[b] = [(1/30) sum_u s[b,u] pf[b,u,:],
                                         (1/6)  sum_u        pf[b,u,:]]  (R^4096)
    W2 = fc_w @ cls_w  (4096x200),  bias = fc_b @ cls_w + cls_b
with s[b,u] = sum_v w[b,u,v] the edge-weight row sums from cdds box centers.

Sharding: the CONTRACTION dim k (=2*D) is split 8 ways. Each core streams the
full batch but only its 256-column d-slice of part_feats (12.6 MB), computes
s for all samples (stage 0, pipelined in 3 column-chunks), forms its 512-row
slice of xT (stage 1), computes its W2 k-slice from a host-pretransposed bf16
fc_w slice (no PE transposes, no AllGather), and accumulates partial outputs
out_part[b, c] over its k-slice (stage 2, sample-major PSUM so partials DMA
straight out). One bf16 ReduceScatter sums the 8 partials and hands each
core its contiguous 256-sample shard; bias is added once after reduction.
"""
import sys
sys.path.insert(0, '/opt/trn_rl_repo')
import numpy as np
import ml_dtypes
import concourse.bass as bass
import concourse.bacc as bacc
import concourse.tile as tile
import concourse.mybir as mybir
from concourse import bass_utils

N_CORES = 8
B_FULL = 2048

F32 = mybir.dt.float32
F32R = mybir.dt.float32r
BF16 = mybir.dt.bfloat16
FP8 = mybir.dt.float8e4
ALU = mybir.AluOpType
ACTF = mybir.ActivationFunctionType
ALPHA = 0.015

D, H, C, NN = 2048, 1024, 200, 6
CP = 256                    # C padded to a 512-byte bf16 line
RT, TS = 126, 21            # rows per sample-tile, samples per sample-tile
DL = D // N_CORES           # d-columns per core (256)
NKT = (2 * DL) // 128       # 4 k-tiles per core
NHT = H // 128              # 8 h-tiles
JC = 33                     # stage-0 pipeline chunk (NJS = 3*JC)


def ap_of(ap, offset, pattern):
    return bass.AP(ap.tensor, offset, pattern)


def ins_bcast(ap, idx, n):
    """Insert a broadcast (step-0) dim into an AP at position idx."""
    a = [list(d) for d in ap.ap]
    a.insert(idx, [0, n])
    return bass.AP(ap.tensor, ap.offset, a)


def bcast_last(ap, n):
    """Replace a singleton last dim with a step-0 broadcast of size n."""
    a = [list(d) for d in ap.ap]
    assert a[-1][1] == 1, a
    return bass.AP(ap.tensor, ap.offset, a[:-1] + [[0, n]])


def build_nc(B_loc=B_FULL, n_cores=8, **_):
    B = B_FULL                          # full batch on every core
    NJ = -(-B // TS)                    # 98 sample-tiles
    b0s = [TS * j for j in range(NJ - 1)] + [B - TS]
    LO = NJ * TS - B                    # overlap of last stile (10)
    NL = TS - LO                        # new samples in last stile (11)
    NJS = NJ + (1 if LO else 0)         # stage-0 columns (99)
    NBT = B // 128                      # 16 output b-tiles
    assert NJS == 3 * JC
    # pf super-tiles over tiles 0..NJ-2; tile NJ-1 (the overlap tail) is solo
    STJ = 7
    sts = []
    j0 = 0
    while j0 < NJ - 1 - 7:
        J = min(STJ, NJ - 1 - 7 - j0)
        sts.append((j0, J))
        j0 += J
    for J in (4, 2, 1):
        sts.append((j0, J))
        j0 += J
    assert j0 == NJ - 1

    nc = bacc.Bacc("TRN2", target_bir_lowering=False, debug=False,
                   enable_asserts=True, num_devices=n_cores,
                   dynamic_dma_scratch_size=65536)
    pf = nc.dram_tensor("pf", [B * NN, DL], F32, kind="ExternalInput").ap()
    # box coords pre-gathered on host into stage-0's partition layout:
    # cdds4[p=(t,u), j, 0:4] = cdds[b0s[j] + t, 1+6*u : 5+6*u]
    cdds4 = nc.dram_tensor("cdds4", [RT, NJS * 4], F32,
                           kind="ExternalInput").ap()
    fcwT = nc.dram_tensor("fcwT", [H, 2 * DL], BF16, kind="ExternalInput").ap()
    clsw = nc.dram_tensor("clsw", [H, CP], BF16, kind="ExternalInput").ap()
    biasr = nc.dram_tensor("biasr", [1, C], F32, kind="ExternalInput").ap()
    # small f32 stage-0 constants in one tensor/DMA: [sel6 | mask_c | ma | mp]
    NCC = 6 + 6 + TS + TS
    consts = nc.dram_tensor("consts", [RT, NCC], F32, kind="ExternalInput").ap()
    gsum = nc.dram_tensor("gsum", [RT, RT], F32, kind="ExternalInput").ap()
    out = nc.dram_tensor("out", [2 * 128, C], F32, kind="ExternalOutput").ap()

    with tile.TileContext(nc) as tc:
        with tc.tile_pool(name="persist", bufs=1) as pp, \
             tc.tile_pool(name="dram", bufs=1, space="DRAM") as dp:

            # ---------------- persistent SBUF ----------------
            xT = pp.tile([128, NKT * B], BF16)           # stage-2 lhsT source
            wall = pp.tile([RT, NJS * 42], F32)          # stage-1 rhs (block diag)
            fcw_sb = pp.tile([128, NHT * 2 * DL], BF16)  # [h%128, ht, k]
            clsw_sb = pp.tile([128, NHT * CP], BF16)     # [h%128, ht, c]
            w2b = pp.tile([128, NKT * CP], BF16)         # [k%128, kt, c]
            bias_sb = pp.tile([1, C], F32)
            bias_bc = pp.tile([128, C], F32)
            ones_sb = pp.tile([1, 128], F32)
            scr_act = pp.tile([1, 8], F32)
            c_all = pp.tile([RT, NCC], F32)
            c_sel6 = c_all[:, 0:6]
            c_maskc = c_all[:, 6:12]
            c_ma21 = c_all[:, 12:12 + TS]
            c_mp21 = c_all[:, 12 + TS:NCC]
            # stage-0 working set
            own4 = pp.tile([RT, NJS * 4], F32)
            sxy = pp.tile([RT, NJS * 2], F32)
            rhs_all = pp.tile([RT, NJS * 12], F32)
            all_xy = pp.tile([RT, NJS * 12], F32)
            dall = pp.tile([RT, NJS * 6], F32)           # dx, d2, dist, em_minus
            dall2 = pp.tile([RT, NJS * 6], F32)          # dy, then relu scratch
            em = pp.tile([RT, NJS * 6], F32)
            esum = pp.tile([RT, NJS], F32)
            mean_sb = pp.tile([RT, NJS], F32)
            s_col = pp.tile([RT, NJS], F32)

            pts_all = pp.tile([128, NBT * CP], BF16)
            b_in = dp.tile([B, CP], BF16)
            b_out = dp.tile([256, CP], BF16)

            # -------- DMAs: consts + cdds4 + weights ahead of the pf stream
            c_gsum = pp.tile([RT, RT], F32)
            with tc.high_priority():
                nc.sync.dma_start(own4[:], cdds4)
                nc.sync.dma_start(c_gsum[:], gsum)
                nc.scalar.dma_start(c_all[:], consts)
                nc.scalar.dma_start(bias_sb[:], biasr)
                nc.sync.dma_start(
                    fcw_sb[:].rearrange("p (ht k) -> p ht k", k=2 * DL),
                    fcwT.rearrange("(ht p) k -> p ht k", p=128))
                nc.sync.dma_start(
                    clsw_sb[:].rearrange("p (ht c) -> p ht c", c=CP),
                    clsw.rearrange("(ht p) c -> p ht c", p=128))
            # ACT table prewarm for Sqrt (Exp's load stays on the chain once)
            nc.gpsimd.memset(scr_act[:], 1.0)
            nc.scalar.activation(scr_act[:], scr_act[:], ACTF.Sqrt, scale=1.0)
            nc.gpsimd.memset(ones_sb[:], 1.0)
            # wall p-part is constant: wall[:, j, 21:42] = mp21 (all j)
            wv = wall[:].rearrange("p (j f) -> p j f", f=42)
            nc.gpsimd.tensor_copy(
                ap_of(wall[:], 21, [[NJS * 42, RT], [42, NJS], [1, TS]]),
                ins_bcast(c_mp21, 1, NJS))

            # ---------------- stage 0: edge weights (3 chunks) ------------
            o4 = own4[:].rearrange("p (j f) -> p j f", f=4)
            sx2 = sxy[:].rearrange("p (j f) -> p j f", f=2)
            r12 = rhs_all[:].rearrange("p (j f) -> p j f", f=12)
            a12 = all_xy[:].rearrange("p (j f) -> p j f", f=12)
            d6 = dall[:].rearrange("p (j f) -> p j f", f=6)
            e6 = dall2[:].rearrange("p (j f) -> p j f", f=6)
            m6 = em[:].rearrange("p (j f) -> p j f", f=6)
            CH = [slice(q * JC, (q + 1) * JC) for q in range(3)]

            def sel_(q):
                return ins_bcast(c_sel6, 1, JC)

            for q, s in enumerate(CH):
                nc.vector.tensor_add(sx2[:, s, 1:2], o4[:, s, 0:1],
                                     o4[:, s, 2:3])
                nc.vector.tensor_add(sx2[:, s, 0:1], o4[:, s, 1:2],
                                     o4[:, s, 3:4])
                nc.vector.tensor_mul(r12[:, s, 0:6], sel_(q),
                                     bcast_last(sx2[:, s, 0:1], 6))
                nc.vector.tensor_mul(r12[:, s, 6:12], sel_(q),
                                     bcast_last(sx2[:, s, 1:2], 6))

            with tc.tile_pool(name="ps0", bufs=1, space="PSUM") as ps0:
                for q, s in enumerate(CH):
                    gch = ps0.tile([RT, JC * 12], F32, tag="gch", bufs=2)
                    nc.tensor.matmul(gch[:], c_gsum[:],
                                     rhs_all[:, q * JC * 12:(q + 1) * JC * 12],
                                     start=True, stop=True)
                    nc.vector.tensor_copy(
                        all_xy[:, q * JC * 12:(q + 1) * JC * 12], gch[:])
                for q, s in enumerate(CH):
                    sx_b = bcast_last(sx2[:, s, 0:1], 6)
                    sy_b = bcast_last(sx2[:, s, 1:2], 6)
                    nc.gpsimd.tensor_sub(e6[:, s, :], sy_b, a12[:, s, 6:12])
                    nc.gpsimd.tensor_mul(e6[:, s, :], e6[:, s, :], e6[:, s, :])
                    nc.vector.tensor_sub(d6[:, s, :], sx_b, a12[:, s, 0:6])
                    nc.vector.tensor_mul(d6[:, s, :], d6[:, s, :], d6[:, s, :])
                    nc.vector.tensor_add(d6[:, s, :], d6[:, s, :], e6[:, s, :])
                # single sqrt + single exp (chunking would thrash the ACT
                # function table: 1.28us reload per Sqrt<->Exp switch)
                nc.scalar.activation(dall[:], dall[:], ACTF.Sqrt, scale=0.25)
                nc.scalar.activation(dall[:], dall[:], ACTF.Exp, scale=-ALPHA)
                for q, s in enumerate(CH):
                    # em = exp * mask ; esum = sum_v em
                    nc.vector.tensor_mul(m6[:, s, :], d6[:, s, :],
                                         ins_bcast(c_maskc, 1, JC))
                    nc.vector.tensor_reduce(
                        esum[:, s], m6[:, s, :], mybir.AxisListType.X,
                        ALU.add)
                for q, s in enumerate(CH):
                    mps = ps0.tile([RT, JC], F32, tag="mps", bufs=2)
                    nc.tensor.matmul(mps[:], c_gsum[:], esum[:, s],
                                     start=True, stop=True)
                    nc.vector.tensor_copy(mean_sb[:, s], mps[:])
                for q, s in enumerate(CH):
                    # em_minus = em - mean/30 ; s' = sum_v relu(em_minus)
                    # (the 0.8 = SCALE/30 factor is folded into ma21)
                    nc.vector.scalar_tensor_tensor(
                        d6[:, s, :], ins_bcast(mean_sb[:, s], 2, 6),
                        -1.0 / 30.0, m6[:, s, :], op0=ALU.mult, op1=ALU.add)
                    nc.vector.tensor_relu(e6[:, s, :], d6[:, s, :])
                    nc.vector.tensor_reduce(s_col[:, s], e6[:, s, :],
                                            mybir.AxisListType.X, ALU.add)
                    # wall a-part: wall[:, j, 0:21] = (0.8*ma21) * s'[:, j]
                    nc.vector.tensor_mul(
                        ap_of(wall[:], q * JC * 42,
                              [[NJS * 42, RT], [42, JC], [1, TS]]),
                        ins_bcast(c_ma21, 1, JC),
                        ins_bcast(s_col[:, s], 2, TS))

            # ---------------- W2 k-slice + bias broadcast ----------------
            fS = fcw_sb[:].rearrange("p (ht k) -> p ht k", k=2 * DL)
            cS = clsw_sb[:].rearrange("p (ht c) -> p ht c", c=CP)
            w2v = w2b[:].rearrange("p (kt c) -> p kt c", c=CP)
            with tc.tile_pool(name="psw", bufs=1, space="PSUM") as psw:
                for kt in range(NKT):
                    wps = psw.tile([128, CP], F32, tag="wps", bufs=2)
                    for ht in range(NHT):
                        nc.tensor.matmul(
                            wps[:], fS[:, ht, kt * 128:(kt + 1) * 128],
                            cS[:, ht, :],
                            start=(ht == 0), stop=(ht == NHT - 1))
                    nc.vector.tensor_copy(w2v[:, kt, :], wps[:])
                bps = psw.tile([128, C], F32, tag="bps")
                nc.tensor.matmul(bps[:], ones_sb[:], bias_sb[:], start=True,
                                 stop=True)
                nc.vector.tensor_copy(bias_bc[:], bps[:])

            # ---------------- stage 1 + interleaved stage 2 ----------------
            xv = xT[:].rearrange("p (kt b) -> p kt b", b=B)
            lo, nl = LO, NL
            state = {"bt": 0, "ev": 0, "pt": None}
            with tc.tile_pool(name="pfp", bufs=7) as pfp, \
                 tc.tile_pool(name="ps1", bufs=1, space="PSUM") as ps1, \
                 tc.tile_pool(name="ps2", bufs=1, space="PSUM") as ps2:

                def do_btile(t):
                    ops = ps2.tile([128, CP], F32, tag="ops", bufs=2)
                    for kt in range(NKT):
                        nc.tensor.matmul(
                            ops[:],
                            ap_of(xT[:], kt * B + 128 * t,
                                  [[NKT * B, 128], [1, 128]]),
                            w2v[:, kt, :],
                            start=(kt == 0), stop=(kt == NKT - 1))
                    nc.scalar.copy(pts_all[:, t * CP:(t + 1) * CP], ops[:])

                def post():
                    # shard rows (h*128+p) -> SBUF [p, h, c]; add bias; store
                    pb = pp.tile([128, 2 * CP], BF16, name="pb")
                    pfq = pp.tile([128, 2 * C], F32, name="pfq")
                    nc.sync.dma_start(
                        pb[:].rearrange("p (h c) -> p h c", c=CP),
                        b_out.opt().rearrange("(h p) c -> p h c", p=128))
                    nc.vector.tensor_add(
                        pfq[:].rearrange("p (h c) -> p h c", c=C),
                        ap_of(pb[:], 0, [[2 * CP, 128], [CP, 2], [1, C]]),
                        ins_bcast(bias_bc[:], 1, 2))
                    nc.sync.dma_start(
                        out.rearrange("(h p) c -> p h c", p=128),
                        pfq[:].rearrange("p (h c) -> p h c", c=C))

                def do_tile(j, pft_ap):
                    last = j == NJ - 1 and lo > 0
                    rt = 6 * nl if last else RT
                    ns = nl if last else TS
                    if last:
                        rhs_w = ap_of(wall[:], (NJS - 1) * 42,
                                      [[NJS * 42, rt], [21, 2], [1, ns]])
                    else:
                        rhs_w = wv[:, j, :]
                    psA = ps1.tile([128, 84], F32, tag="psA", bufs=4)
                    for db in range(2):
                        nc.tensor.matmul(
                            psA[:, db * 2 * ns:(db + 1) * 2 * ns],
                            pft_ap[0:rt, db * 128:(db + 1) * 128],
                            rhs_w, start=True, stop=True)
                    c0 = b0s[j] + lo if last else b0s[j]
                    # one 4D copy: psA[p, db, h, s] -> xT[p, (2h+db)*B + c0+s]
                    src = ap_of(psA[:], 0,
                                [[84, 128], [ns, 2], [2 * ns, 2], [1, ns]])
                    dst = ap_of(xT[:], c0,
                                [[NKT * B, 128], [2 * B, 2], [B, 2], [1, ns]])
                    if state["ev"] % 2 == 0:
                        nc.vector.tensor_copy(dst, src)
                    else:
                        nc.scalar.copy(dst, src)
                    state["ev"] += 1
                    if last:
                        return   # tail runs first; it must not advance bt
                    cov = TS * (j + 1)   # samples covered by regular tiles
                    while state["bt"] < NBT and (
                            128 * (state["bt"] + 1) <= cov
                            or (state["bt"] == NBT - 1 and cov >= B - NL)):
                        do_btile(state["bt"])
                        state["bt"] += 1

                pf_tl = pfp.tile([6 * NL, DL], F32, tag="pftail", bufs=1)
                nc.sync.dma_start(pf_tl[:], pf[(B - NL) * 6:B * 6, :])
                do_tile(NJ - 1, pf_tl[:])
                for si, (j0, J) in enumerate(sts):
                    pf_st = pfp.tile([RT, STJ * DL], F32, tag="pf", bufs=7)
                    nc.sync.dma_start(
                        pf_st[:, 0:J * DL].rearrange(
                            "p (jj d) -> p jj d", d=DL),
                        ap_of(pf, j0 * RT * DL,
                              [[DL, RT], [RT * DL, J], [1, DL]]))
                    for jj in range(J):
                        do_tile(j0 + jj, pf_st[:, jj * DL:(jj + 1) * DL])
                while state["bt"] < NBT:
                    do_btile(state["bt"])
                    state["bt"] += 1
                # partials -> b_in: one bulk DMA for slots 0..13 plus two
                # solos, all on the (now idle) SP HWDGE queue
                nc.sync.dma_start(
                    ap_of(b_in.opt(), 0,
                          [[CP, 128], [128 * CP, NBT - 2], [1, CP]]),
                    ap_of(pts_all[:], 0,
                          [[NBT * CP, 128], [CP, NBT - 2], [1, CP]]))
                for t in (NBT - 2, NBT - 1):
                    nc.sync.dma_start(
                        ap_of(b_in.opt(), t * 128 * CP, [[CP, 128], [1, CP]]),
                        pts_all[:, t * CP:(t + 1) * CP])
                nc.gpsimd.collective_compute(
                    "ReduceScatter", ALU.add,
                    replica_groups=[list(range(n_cores))],
                    ins=[b_in.opt()], outs=[b_out.opt()])
                post()
    nc.compile()
    return nc


def make_host_inputs(part_feats, cdds, fc_w, fc_b, cls_w, cls_b, n_cores=8):
    """Shard + prepare per-core in_maps from full inputs."""
    B = part_feats.shape[0]
    p = np.arange(RT)
    maskc = (p[:, None] % 6 != np.arange(6)[None, :]).astype(np.float32)
    sel = (p[:, None] % 6 == np.arange(6)[None, :]).astype(np.float32)
    gs = (p[:, None] // 6 == p[None, :] // 6).astype(np.float32)
    ma = np.zeros((RT, TS), np.float32)
    ma[p, p // 6] = 0.8                  # SCALE/30 folded in
    mp = np.zeros((RT, TS), np.float32)
    mp[p, p // 6] = 1.0 / 6.0
    bias = (fc_b @ cls_w + cls_b).reshape(1, C).astype(np.float32)
    # cdds4[p=(t,u), j, 0:4] = cdds[b0s[j] + t, 1+6u : 5+6u]; tail column
    # (j = NJ) = the last NL samples at partitions 0:6*NL, zero-padded.
    NJ = -(-B // TS)
    LO = NJ * TS - B
    NL = TS - LO
    NJS = NJ + (1 if LO else 0)
    b0s = np.array([TS * j for j in range(NJ - 1)] + [B - TS])
    cd = cdds.reshape(B, 6, 6)[:, :, 1:5]                # (B, u, 4)
    t_i, u_i = p // 6, p % 6
    cdds4 = np.zeros((RT, NJS, 4), np.float32)
    cdds4[:, 0:NJ, :] = cd[b0s[None, :] + t_i[:, None], u_i[:, None], :]
    if LO:
        cdds4[0:6 * NL, NJ, :] = cd[B - NL:B].reshape(6 * NL, 4)
    cdds4 = np.ascontiguousarray(cdds4.reshape(RT, NJS * 4))
    bf = ml_dtypes.bfloat16
    f8 = ml_dtypes.float8_e4m3fn
    clsw_pad = np.zeros((H, CP), np.float32)
    clsw_pad[:, 0:C] = cls_w
    clsw_pad = clsw_pad.astype(bf)
    in_maps = []
    for c in range(n_cores):
        rows = np.r_[c * DL:(c + 1) * DL, D + c * DL:D + (c + 1) * DL]
        in_maps.append({
            "pf": np.ascontiguousarray(
                part_feats[:, :, c * DL:(c + 1) * DL].reshape(B * NN, DL)),
            "cdds4": cdds4,
            "fcwT": np.ascontiguousarray(fc_w[rows, :].T).astype(bf),
            "clsw": clsw_pad,
            "biasr": bias,
            "consts": np.ascontiguousarray(
                np.concatenate([sel, maskc, ma, mp], axis=1)),
            "gsum": gs,
        })
    return in_maps


_NC_CACHE = {}


def kernel(part_feats, cdds, fc_w, fc_b, cls_w, cls_b):
    part_feats = np.ascontiguousarray(part_feats, dtype=np.float32)
    cdds = np.ascontiguousarray(cdds, dtype=np.float32)
    fc_w = np.ascontiguousarray(fc_w, dtype=np.float32)
    fc_b = np.ascontiguousarray(fc_b, dtype=np.float32)
    cls_w = np.ascontiguousarray(cls_w, dtype=np.float32)
    cls_b = np.ascontiguousarray(cls_b, dtype=np.float32)
    B = part_feats.shape[0]
    if "nc" not in _NC_CACHE:
        _NC_CACHE["nc"] = build_nc(n_cores=N_CORES)
    nc = _NC_CACHE["nc"]
    in_maps = make_host_inputs(part_feats, cdds, fc_w, fc_b, cls_w, cls_b,
                               n_cores=N_CORES)
    # First execution after compile has been observed to produce bad output
    # intermittently (runtime warmup); run once to warm up, then take the
    # second execution's result.
    bass_utils.run_bass_kernel_spmd(
        nc, in_maps, core_ids=list(range(N_CORES)))
    res = bass_utils.run_bass_kernel_spmd(
        nc, in_maps, core_ids=list(range(N_CORES)))
    # core c's "out" = samples [256c, 256(c+1))
    return np.concatenate([res.results[c]["out"] for c in range(N_CORES)],
                          axis=0)


# revision 22
# speedup vs baseline: 1.0938x; 1.0014x over previous
"""Trainium2 Bass kernel for nn_CNNtoGraph_77936476553433 (8-core k-parallel).

The GNN collapses algebraically: per sample b
    out[b] = x[b] @ W2 + bias,   x[b] = [(1/30) sum_u s[b,u] pf[b,u,:],
                                         (1/6)  sum_u        pf[b,u,:]]  (R^4096)
    W2 = fc_w @ cls_w  (4096x200),  bias = fc_b @ cls_w + cls_b
with s[b,u] = sum_v w[b,u,v] the edge-weight row sums from cdds box centers.

Sharding: the CONTRACTION dim k (=2*D) is split 8 ways. Each core streams the
full batch but only its 256-column d-slice of part_feats (12.6 MB), computes
s for all samples (stage 0, pipelined in 3 column-chunks), forms its 512-row
slice of xT (stage 1), computes its W2 k-slice from a host-pretransposed bf16
fc_w slice (no PE transposes, no AllGather), and accumulates partial outputs
out_part[b, c] over its k-slice (stage 2, sample-major PSUM so partials DMA
straight out). One bf16 ReduceScatter sums the 8 partials and hands each
core its contiguous 256-sample shard; bias is added once after reduction.
"""
import sys
sys.path.insert(0, '/opt/trn_rl_repo')
import numpy as np
import ml_dtypes
import concourse.bass as bass
import concourse.bacc as bacc
import concourse.tile as tile
import concourse.mybir as mybir
from concourse import bass_utils

N_CORES = 8
B_FULL = 2048

F32 = mybir.dt.float32
F32R = mybir.dt.float32r
BF16 = mybir.dt.bfloat16
FP8 = mybir.dt.float8e4
ALU = mybir.AluOpType
ACTF = mybir.ActivationFunctionType
ALPHA = 0.015

D, H, C, NN = 2048, 1024, 200, 6
CP = 256                    # C padded to a 512-byte bf16 line
RT, TS = 126, 21            # rows per sample-tile, samples per sample-tile
DL = D // N_CORES           # d-columns per core (256)
NKT = (2 * DL) // 128       # 4 k-tiles per core
NHT = H // 128              # 8 h-tiles
JC = 33                     # stage-0 pipeline chunk (NJS = 3*JC)


def ap_of(ap, offset, pattern):
    return bass.AP(ap.tensor, offset, pattern)


def ins_bcast(ap, idx, n):
    """Insert a broadcast (step-0) dim into an AP at position idx."""
    a = [list(d) for d in ap.ap]
    a.insert(idx, [0, n])
    return bass.AP(ap.tensor, ap.offset, a)


def bcast_last(ap, n):
    """Replace a singleton last dim with a step-0 broadcast of size n."""
    a = [list(d) for d in ap.ap]
    assert a[-1][1] == 1, a
    return bass.AP(ap.tensor, ap.offset, a[:-1] + [[0, n]])


def build_nc(B_loc=B_FULL, n_cores=8, **_):
    B = B_FULL                          # full batch on every core
    NJ = -(-B // TS)                    # 98 sample-tiles
    b0s = [TS * j for j in range(NJ - 1)] + [B - TS]
    LO = NJ * TS - B                    # overlap of last stile (10)
    NL = TS - LO                        # new samples in last stile (11)
    NJS = NJ + (1 if LO else 0)         # stage-0 columns (99)
    NBT = B // 128                      # 16 output b-tiles
    assert NJS == 3 * JC
    # pf super-tiles over tiles 0..NJ-2; tile NJ-1 (the overlap tail) is solo
    STJ = 7
    sts = []
    j0 = 0
    while j0 < NJ - 1 - 7:
        J = min(STJ, NJ - 1 - 7 - j0)
        sts.append((j0, J))
        j0 += J
    for J in (4, 2, 1):
        sts.append((j0, J))
        j0 += J
    assert j0 == NJ - 1

    nc = bacc.Bacc("TRN2", target_bir_lowering=False, debug=False,
                   enable_asserts=True, num_devices=n_cores,
                   dynamic_dma_scratch_size=65536)
    pf = nc.dram_tensor("pf", [B * NN, DL], F32, kind="ExternalInput").ap()
    # box coords pre-gathered on host into stage-0's partition layout:
    # cdds4[p=(t,u), j, 0:4] = cdds[b0s[j] + t, 1+6*u : 5+6*u]
    cdds4 = nc.dram_tensor("cdds4", [RT, NJS * 4], F32,
                           kind="ExternalInput").ap()
    fcwT = nc.dram_tensor("fcwT", [H, 2 * DL], BF16, kind="ExternalInput").ap()
    clsw = nc.dram_tensor("clsw", [H, CP], BF16, kind="ExternalInput").ap()
    biasr = nc.dram_tensor("biasr", [1, C], F32, kind="ExternalInput").ap()
    # small f32 stage-0 constants in one tensor/DMA: [sel6 | mask_c | ma | mp]
    NCC = 6 + 6 + TS + TS
    consts = nc.dram_tensor("consts", [RT, NCC], F32, kind="ExternalInput").ap()
    gsum = nc.dram_tensor("gsum", [RT, RT], F32, kind="ExternalInput").ap()
    out = nc.dram_tensor("out", [2 * 128, C], F32, kind="ExternalOutput").ap()

    with tile.TileContext(nc) as tc:
        with tc.tile_pool(name="persist", bufs=1) as pp, \
             tc.tile_pool(name="dram", bufs=1, space="DRAM") as dp:

            # ---------------- persistent SBUF ----------------
            xT = pp.tile([128, NKT * B], BF16)           # stage-2 lhsT source
            wall = pp.tile([RT, NJS * 42], F32)          # stage-1 rhs (block diag)
            fcw_sb = pp.tile([128, NHT * 2 * DL], BF16)  # [h%128, ht, k]
            clsw_sb = pp.tile([128, NHT * CP], BF16)     # [h%128, ht, c]
            w2b = pp.tile([128, NKT * CP], BF16)         # [k%128, kt, c]
            bias_sb = pp.tile([1, C], F32)
            bias_bc = pp.tile([128, C], F32)
            ones_sb = pp.tile([1, 128], F32)
            scr_act = pp.tile([1, 8], F32)
            c_all = pp.tile([RT, NCC], F32)
            c_sel6 = c_all[:, 0:6]
            c_maskc = c_all[:, 6:12]
            c_ma21 = c_all[:, 12:12 + TS]
            c_mp21 = c_all[:, 12 + TS:NCC]
            # stage-0 working set
            own4 = pp.tile([RT, NJS * 4], F32)
            sxy = pp.tile([RT, NJS * 2], F32)
            rhs_all = pp.tile([RT, NJS * 12], F32)
            all_xy = pp.tile([RT, NJS * 12], F32)
            dall = pp.tile([RT, NJS * 6], F32)           # dx, d2, dist, em_minus
            dall2 = pp.tile([RT, NJS * 6], F32)          # dy, then relu scratch
            em = pp.tile([RT, NJS * 6], F32)
            esum = pp.tile([RT, NJS], F32)
            mean_sb = pp.tile([RT, NJS], F32)
            s_col = pp.tile([RT, NJS], F32)

            pts_all = pp.tile([128, NBT * CP], BF16)
            b_in = dp.tile([B, CP], BF16)
            b_out = dp.tile([256, CP], BF16)

            # -------- DMAs: consts + cdds4 + weights ahead of the pf stream
            c_gsum = pp.tile([RT, RT], F32)
            with tc.high_priority():
                nc.sync.dma_start(own4[:], cdds4)
                nc.sync.dma_start(c_gsum[:], gsum)
                nc.scalar.dma_start(c_all[:], consts)
                nc.scalar.dma_start(bias_sb[:], biasr)
                nc.sync.dma_start(
                    fcw_sb[:].rearrange("p (ht k) -> p ht k", k=2 * DL),
                    fcwT.rearrange("(ht p) k -> p ht k", p=128))
                nc.sync.dma_start(
                    clsw_sb[:].rearrange("p (ht c) -> p ht c", c=CP),
                    clsw.rearrange("(ht p) c -> p ht c", p=128))
            # ACT table prewarm for Sqrt (Exp's load stays on the chain once)
            nc.gpsimd.memset(scr_act[:], 1.0)
            nc.scalar.activation(scr_act[:], scr_act[:], ACTF.Sqrt, scale=1.0)
            nc.gpsimd.memset(ones_sb[:], 1.0)
            # wall p-part is constant: wall[:, j, 21:42] = mp21 (all j)
            wv = wall[:].rearrange("p (j f) -> p j f", f=42)
            nc.gpsimd.tensor_copy(
                ap_of(wall[:], 21, [[NJS * 42, RT], [42, NJS], [1, TS]]),
                ins_bcast(c_mp21, 1, NJS))

            # ---------------- stage 0: edge weights (3 chunks) ------------
            o4 = own4[:].rearrange("p (j f) -> p j f", f=4)
            sx2 = sxy[:].rearrange("p (j f) -> p j f", f=2)
            r12 = rhs_all[:].rearrange("p (j f) -> p j f", f=12)
            a12 = all_xy[:].rearrange("p (j f) -> p j f", f=12)
            d6 = dall[:].rearrange("p (j f) -> p j f", f=6)
            e6 = dall2[:].rearrange("p (j f) -> p j f", f=6)
            m6 = em[:].rearrange("p (j f) -> p j f", f=6)
            CH = [slice(q * JC, (q + 1) * JC) for q in range(3)]

            def sel_(q):
                return ins_bcast(c_sel6, 1, JC)

            for q, s in enumerate(CH):
                nc.vector.tensor_add(sx2[:, s, 1:2], o4[:, s, 0:1],
                                     o4[:, s, 2:3])
                nc.vector.tensor_add(sx2[:, s, 0:1], o4[:, s, 1:2],
                                     o4[:, s, 3:4])
                nc.vector.tensor_mul(r12[:, s, 0:6], sel_(q),
                                     bcast_last(sx2[:, s, 0:1], 6))
                nc.vector.tensor_mul(r12[:, s, 6:12], sel_(q),
                                     bcast_last(sx2[:, s, 1:2], 6))

            with tc.tile_pool(name="ps0", bufs=1, space="PSUM") as ps0:
                for q, s in enumerate(CH):
                    gch = ps0.tile([RT, JC * 12], F32, tag="gch", bufs=2)
                    nc.tensor.matmul(gch[:], c_gsum[:],
                                     rhs_all[:, q * JC * 12:(q + 1) * JC * 12],
                                     start=True, stop=True)
                    nc.vector.tensor_copy(
                        all_xy[:, q * JC * 12:(q + 1) * JC * 12], gch[:])
                for q, s in enumerate(CH):
                    sx_b = bcast_last(sx2[:, s, 0:1], 6)
                    sy_b = bcast_last(sx2[:, s, 1:2], 6)
                    nc.gpsimd.tensor_sub(e6[:, s, :], sy_b, a12[:, s, 6:12])
                    nc.gpsimd.tensor_mul(e6[:, s, :], e6[:, s, :], e6[:, s, :])
                    nc.vector.tensor_sub(d6[:, s, :], sx_b, a12[:, s, 0:6])
                    nc.vector.tensor_mul(d6[:, s, :], d6[:, s, :], d6[:, s, :])
                    nc.vector.tensor_add(d6[:, s, :], d6[:, s, :], e6[:, s, :])
                # single sqrt + single exp (chunking would thrash the ACT
                # function table: 1.28us reload per Sqrt<->Exp switch)
                nc.scalar.activation(dall[:], dall[:], ACTF.Sqrt, scale=0.25)
                nc.scalar.activation(dall[:], dall[:], ACTF.Exp, scale=-ALPHA)
                for q, s in enumerate(CH):
                    # em = exp * mask ; esum = sum_v em
                    nc.vector.tensor_mul(m6[:, s, :], d6[:, s, :],
                                         ins_bcast(c_maskc, 1, JC))
                    nc.vector.tensor_reduce(
                        esum[:, s], m6[:, s, :], mybir.AxisListType.X,
                        ALU.add)
                for q, s in enumerate(CH):
                    mps = ps0.tile([RT, JC], F32, tag="mps", bufs=2)
                    nc.tensor.matmul(mps[:], c_gsum[:], esum[:, s],
                                     start=True, stop=True)
                    nc.vector.tensor_copy(mean_sb[:, s], mps[:])
                for q, s in enumerate(CH):
                    # em_minus = em - mean/30 ; s' = sum_v relu(em_minus)
                    # (the 0.8 = SCALE/30 factor is folded into ma21)
                    nc.vector.scalar_tensor_tensor(
                        d6[:, s, :], ins_bcast(mean_sb[:, s], 2, 6),
                        -1.0 / 30.0, m6[:, s, :], op0=ALU.mult, op1=ALU.add)
                    nc.vector.tensor_relu(e6[:, s, :], d6[:, s, :])
                    nc.vector.tensor_reduce(s_col[:, s], e6[:, s, :],
                                            mybir.AxisListType.X, ALU.add)
                    # wall a-part: wall[:, j, 0:21] = (0.8*ma21) * s'[:, j]
                    nc.vector.tensor_mul(
                        ap_of(wall[:], q * JC * 42,
                              [[NJS * 42, RT], [42, JC], [1, TS]]),
                        ins_bcast(c_ma21, 1, JC),
                        ins_bcast(s_col[:, s], 2, TS))

            # ---------------- W2 k-slice + bias broadcast ----------------
            fS = fcw_sb[:].rearrange("p (ht k) -> p ht k", k=2 * DL)
            cS = clsw_sb[:].rearrange("p (ht c) -> p ht c", c=CP)
            w2v = w2b[:].rearrange("p (kt c) -> p kt c", c=CP)
            with tc.tile_pool(name="psw", bufs=1, space="PSUM") as psw:
                for kt in range(NKT):
                    wps = psw.tile([128, CP], F32, tag="wps", bufs=2)
                    for ht in range(NHT):
                        nc.tensor.matmul(
                            wps[:], fS[:, ht, kt * 128:(kt + 1) * 128],
                            cS[:, ht, :],
                            start=(ht == 0), stop=(ht == NHT - 1))
                    nc.vector.tensor_copy(w2v[:, kt, :], wps[:])
                bps = psw.tile([128, C], F32, tag="bps")
                nc.tensor.matmul(bps[:], ones_sb[:], bias_sb[:], start=True,
                                 stop=True)
                nc.vector.tensor_copy(bias_bc[:], bps[:])

            # ---------------- stage 1 + interleaved stage 2 ----------------
            xv = xT[:].rearrange("p (kt b) -> p kt b", b=B)
            lo, nl = LO, NL
            state = {"bt": 0, "ev": 0, "pt": None}
            with tc.tile_pool(name="pfp", bufs=7) as pfp, \
                 tc.tile_pool(name="ps1", bufs=1, space="PSUM") as ps1, \
                 tc.tile_pool(name="ps2", bufs=1, space="PSUM") as ps2:

                def do_btile(t):
                    ops = ps2.tile([128, CP], F32, tag="ops", bufs=2)
                    for kt in range(NKT):
                        nc.tensor.matmul(
                            ops[:],
                            ap_of(xT[:], kt * B + 128 * t,
                                  [[NKT * B, 128], [1, 128]]),
                            w2v[:, kt, :],
                            start=(kt == 0), stop=(kt == NKT - 1))
                    if t >= NBT - 4:
                        nc.vector.tensor_copy(
                            pts_all[:, t * CP:(t + 1) * CP], ops[:])
                    else:
                        nc.scalar.copy(pts_all[:, t * CP:(t + 1) * CP],
                                       ops[:])

                def post():
                    # shard rows (h*128+p) -> SBUF [p, h, c]; add bias; store
                    pb = pp.tile([128, 2 * CP], BF16, name="pb")
                    pfq = pp.tile([128, 2 * C], F32, name="pfq")
                    nc.sync.dma_start(
                        pb[:].rearrange("p (h c) -> p h c", c=CP),
                        b_out.opt().rearrange("(h p) c -> p h c", p=128))
                    nc.vector.tensor_add(
                        pfq[:].rearrange("p (h c) -> p h c", c=C),
                        ap_of(pb[:], 0, [[2 * CP, 128], [CP, 2], [1, C]]),
                        ins_bcast(bias_bc[:], 1, 2))
                    nc.sync.dma_start(
                        out.rearrange("(h p) c -> p h c", p=128),
                        pfq[:].rearrange("p (h c) -> p h c", c=C))

                def do_tile(j, pft_ap):
                    last = j == NJ - 1 and lo > 0
                    rt = 6 * nl if last else RT
                    ns = nl if last else TS
                    if last:
                        rhs_w = ap_of(wall[:], (NJS - 1) * 42,
                                      [[NJS * 42, rt], [21, 2], [1, ns]])
                    else:
                        rhs_w = wv[:, j, :]
                    psA = ps1.tile([128, 84], F32, tag="psA", bufs=4)
                    for db in range(2):
                        nc.tensor.matmul(
                            psA[:, db * 2 * ns:(db + 1) * 2 * ns],
                            pft_ap[0:rt, db * 128:(db + 1) * 128],
                            rhs_w, start=True, stop=True)
                    c0 = b0s[j] + lo if last else b0s[j]
                    # one 4D copy: psA[p, db, h, s] -> xT[p, (2h+db)*B + c0+s]
                    src = ap_of(psA[:], 0,
                                [[84, 128], [ns, 2], [2 * ns, 2], [1, ns]])
                    dst = ap_of(xT[:], c0,
                                [[NKT * B, 128], [2 * B, 2], [B, 2], [1, ns]])
                    if state["ev"] % 2 == 0:
                        nc.vector.tensor_copy(dst, src)
                    else:
                        nc.scalar.copy(dst, src)
                    state["ev"] += 1
                    if last:
                        return   # tail runs first; it must not advance bt
                    cov = TS * (j + 1)   # samples covered by regular tiles
                    while state["bt"] < NBT and (
                            128 * (state["bt"] + 1) <= cov
                            or (state["bt"] == NBT - 1 and cov >= B - NL)):
                        do_btile(state["bt"])
                        state["bt"] += 1

                pf_tl = pfp.tile([6 * NL, DL], F32, tag="pftail", bufs=1)
                nc.sync.dma_start(pf_tl[:], pf[(B - NL) * 6:B * 6, :])
                do_tile(NJ - 1, pf_tl[:])
                for si, (j0, J) in enumerate(sts):
                    pf_st = pfp.tile([RT, STJ * DL], F32, tag="pf", bufs=7)
                    nc.sync.dma_start(
                        pf_st[:, 0:J * DL].rearrange(
                            "p (jj d) -> p jj d", d=DL),
                        ap_of(pf, j0 * RT * DL,
                              [[DL, RT], [RT * DL, J], [1, DL]]))
                    for jj in range(J):
                        do_tile(j0 + jj, pf_st[:, jj * DL:(jj + 1) * DL])
                while state["bt"] < NBT:
                    do_btile(state["bt"])
                    state["bt"] += 1
                # partials -> b_in: one bulk DMA for slots 0..13 plus two
                # solos, all on the (now idle) SP HWDGE queue
                nc.sync.dma_start(
                    ap_of(b_in.opt(), 0,
                          [[CP, 128], [128 * CP, NBT - 2], [1, CP]]),
                    ap_of(pts_all[:], 0,
                          [[NBT * CP, 128], [CP, NBT - 2], [1, CP]]))
                for t in (NBT - 2, NBT - 1):
                    nc.sync.dma_start(
                        ap_of(b_in.opt(), t * 128 * CP, [[CP, 128], [1, CP]]),
                        pts_all[:, t * CP:(t + 1) * CP])
                nc.gpsimd.collective_compute(
                    "ReduceScatter", ALU.add,
                    replica_groups=[list(range(n_cores))],
                    ins=[b_in.opt()], outs=[b_out.opt()])
                post()
    nc.compile()
    return nc


def make_host_inputs(part_feats, cdds, fc_w, fc_b, cls_w, cls_b, n_cores=8):
    """Shard + prepare per-core in_maps from full inputs."""
    B = part_feats.shape[0]
    p = np.arange(RT)
    maskc = (p[:, None] % 6 != np.arange(6)[None, :]).astype(np.float32)
    sel = (p[:, None] % 6 == np.arange(6)[None, :]).astype(np.float32)
    gs = (p[:, None] // 6 == p[None, :] // 6).astype(np.float32)
    ma = np.zeros((RT, TS), np.float32)
    ma[p, p // 6] = 0.8                  # SCALE/30 folded in
    mp = np.zeros((RT, TS), np.float32)
    mp[p, p // 6] = 1.0 / 6.0
    bias = (fc_b @ cls_w + cls_b).reshape(1, C).astype(np.float32)
    # cdds4[p=(t,u), j, 0:4] = cdds[b0s[j] + t, 1+6u : 5+6u]; tail column
    # (j = NJ) = the last NL samples at partitions 0:6*NL, zero-padded.
    NJ = -(-B // TS)
    LO = NJ * TS - B
    NL = TS - LO
    NJS = NJ + (1 if LO else 0)
    b0s = np.array([TS * j for j in range(NJ - 1)] + [B - TS])
    cd = cdds.reshape(B, 6, 6)[:, :, 1:5]                # (B, u, 4)
    t_i, u_i = p // 6, p % 6
    cdds4 = np.zeros((RT, NJS, 4), np.float32)
    cdds4[:, 0:NJ, :] = cd[b0s[None, :] + t_i[:, None], u_i[:, None], :]
    if LO:
        cdds4[0:6 * NL, NJ, :] = cd[B - NL:B].reshape(6 * NL, 4)
    cdds4 = np.ascontiguousarray(cdds4.reshape(RT, NJS * 4))
    bf = ml_dtypes.bfloat16
    f8 = ml_dtypes.float8_e4m3fn
    clsw_pad = np.zeros((H, CP), np.float32)
    clsw_pad[:, 0:C] = cls_w
    clsw_pad = clsw_pad.astype(bf)
    in_maps = []
    for c in range(n_cores):
        rows = np.r_[c * DL:(c + 1) * DL, D + c * DL:D + (c + 1) * DL]
        in_maps.append({
            "pf": np.ascontiguousarray(
                part_feats[:, :, c * DL:(c + 1) * DL].reshape(B * NN, DL)),
            "cdds4": cdds4,
            "fcwT": np.ascontiguousarray(fc_w[rows, :].T).astype(bf),
            "clsw": clsw_pad,
            "biasr": bias,
            "consts": np.ascontiguousarray(
                np.concatenate([sel, maskc, ma, mp], axis=1)),
            "gsum": gs,
        })
    return in_maps


_NC_CACHE = {}


def kernel(part_feats, cdds, fc_w, fc_b, cls_w, cls_b):
    part_feats = np.ascontiguousarray(part_feats, dtype=np.float32)
    cdds = np.ascontiguousarray(cdds, dtype=np.float32)
    fc_w = np.ascontiguousarray(fc_w, dtype=np.float32)
    fc_b = np.ascontiguousarray(fc_b, dtype=np.float32)
    cls_w = np.ascontiguousarray(cls_w, dtype=np.float32)
    cls_b = np.ascontiguousarray(cls_b, dtype=np.float32)
    B = part_feats.shape[0]
    if "nc" not in _NC_CACHE:
        _NC_CACHE["nc"] = build_nc(n_cores=N_CORES)
    nc = _NC_CACHE["nc"]
    in_maps = make_host_inputs(part_feats, cdds, fc_w, fc_b, cls_w, cls_b,
                               n_cores=N_CORES)
    # First execution after compile has been observed to produce bad output
    # intermittently (runtime warmup); run once to warm up, then take the
    # second execution's result.
    bass_utils.run_bass_kernel_spmd(
        nc, in_maps, core_ids=list(range(N_CORES)))
    res = bass_utils.run_bass_kernel_spmd(
        nc, in_maps, core_ids=list(range(N_CORES)))
    # core c's "out" = samples [256c, 256(c+1))
    return np.concatenate([res.results[c]["out"] for c in range(N_CORES)],
                          axis=0)


# revision 28
# speedup vs baseline: 1.1352x; 1.0378x over previous
"""Trainium2 Bass kernel for nn_CNNtoGraph_77936476553433 (8-core k-parallel).

The GNN collapses algebraically: per sample b
    out[b] = x[b] @ W2 + bias,   x[b] = [(1/30) sum_u s[b,u] pf[b,u,:],
                                         (1/6)  sum_u        pf[b,u,:]]  (R^4096)
    W2 = fc_w @ cls_w  (4096x200),  bias = fc_b @ cls_w + cls_b
with s[b,u] = sum_v w[b,u,v] the edge-weight row sums from cdds box centers.

Sharding: the CONTRACTION dim k (=2*D) is split 8 ways. Each core streams the
full batch but only its 256-column d-slice of part_feats (12.6 MB), computes
s for all samples (stage 0, pipelined in 3 column-chunks), forms its 512-row
slice of xT (stage 1), computes its W2 k-slice from a host-pretransposed bf16
fc_w slice (no PE transposes, no AllGather), and accumulates partial outputs
out_part[b, c] over its k-slice (stage 2, sample-major PSUM so partials DMA
straight out). One bf16 ReduceScatter sums the 8 partials and hands each
core its contiguous 256-sample shard; bias is added once after reduction.
"""
import sys
sys.path.insert(0, '/opt/trn_rl_repo')
import numpy as np
import ml_dtypes
import concourse.bass as bass
import concourse.bacc as bacc
import concourse.tile as tile
import concourse.mybir as mybir
from concourse import bass_utils

N_CORES = 8
B_FULL = 2048

F32 = mybir.dt.float32
F32R = mybir.dt.float32r
BF16 = mybir.dt.bfloat16
FP8 = mybir.dt.float8e4
ALU = mybir.AluOpType
ACTF = mybir.ActivationFunctionType
ALPHA = 0.015

D, H, C, NN = 2048, 1024, 200, 6
CP = 256                    # C padded to a 512-byte bf16 line
RT, TS = 126, 21            # rows per sample-tile, samples per sample-tile
DL = D // N_CORES           # d-columns per core (256)
NKT = (2 * DL) // 128       # 4 k-tiles per core
NHT = H // 128              # 8 h-tiles
JC = 33                     # stage-0 pipeline chunk (NJS = 3*JC)


def ap_of(ap, offset, pattern):
    return bass.AP(ap.tensor, offset, pattern)


def ins_bcast(ap, idx, n):
    """Insert a broadcast (step-0) dim into an AP at position idx."""
    a = [list(d) for d in ap.ap]
    a.insert(idx, [0, n])
    return bass.AP(ap.tensor, ap.offset, a)


def bcast_last(ap, n):
    """Replace a singleton last dim with a step-0 broadcast of size n."""
    a = [list(d) for d in ap.ap]
    assert a[-1][1] == 1, a
    return bass.AP(ap.tensor, ap.offset, a[:-1] + [[0, n]])


def build_nc(B_loc=B_FULL, n_cores=8, **_):
    B = B_FULL                          # full batch on every core
    NJ = -(-B // TS)                    # 98 sample-tiles
    b0s = [TS * j for j in range(NJ - 1)] + [B - TS]
    LO = NJ * TS - B                    # overlap of last stile (10)
    NL = TS - LO                        # new samples in last stile (11)
    NJS = NJ + (1 if LO else 0)         # stage-0 columns (99)
    NBT = B // 128                      # 16 output b-tiles
    assert NJS == 3 * JC
    # pf super-tiles over tiles 0..NJ-2; tile NJ-1 (the overlap tail) is solo
    STJ = 7
    sts = []
    j0 = 0
    while j0 < NJ - 1 - 7:
        J = min(STJ, NJ - 1 - 7 - j0)
        sts.append((j0, J))
        j0 += J
    for J in (4, 2, 1):
        sts.append((j0, J))
        j0 += J
    assert j0 == NJ - 1

    nc = bacc.Bacc("TRN2", target_bir_lowering=False, debug=False,
                   enable_asserts=True, num_devices=n_cores,
                   dynamic_dma_scratch_size=65536)
    pf = nc.dram_tensor("pf", [B * NN, DL], F32, kind="ExternalInput").ap()
    # box coords pre-gathered on host into stage-0's partition layout:
    # cdds4[p=(t,u), j, 0:4] = cdds[b0s[j] + t, 1+6*u : 5+6*u]
    cdds4 = nc.dram_tensor("cdds4", [RT, NJS * 4], F32,
                           kind="ExternalInput").ap()
    fcwT = nc.dram_tensor("fcwT", [H, 2 * DL], BF16, kind="ExternalInput").ap()
    clsw = nc.dram_tensor("clsw", [H, CP], BF16, kind="ExternalInput").ap()
    biasr = nc.dram_tensor("biasr", [1, C], F32, kind="ExternalInput").ap()
    # small f32 stage-0 constants in one tensor/DMA: [sel6 | mask_c | ma | mp]
    NCC = 6 + 6 + TS + TS
    consts = nc.dram_tensor("consts", [RT, NCC], F32, kind="ExternalInput").ap()
    gsum = nc.dram_tensor("gsum", [RT, RT], F32, kind="ExternalInput").ap()
    out = nc.dram_tensor("out", [2 * 128, C], F32, kind="ExternalOutput").ap()

    with tile.TileContext(nc) as tc:
        with tc.tile_pool(name="persist", bufs=1) as pp, \
             tc.tile_pool(name="dram", bufs=1, space="DRAM") as dp:

            # ---------------- persistent SBUF ----------------
            xT = pp.tile([128, NKT * B], BF16)           # stage-2 lhsT source
            wall = pp.tile([RT, NJS * 42], F32)          # stage-1 rhs (block diag)
            fcw_sb = pp.tile([128, NHT * 2 * DL], BF16)  # [h%128, ht, k]
            clsw_sb = pp.tile([128, NHT * CP], BF16)     # [h%128, ht, c]
            w2b = pp.tile([128, NKT * CP], BF16)         # [k%128, kt, c]
            bias_sb = pp.tile([1, C], F32)
            bias_bc = pp.tile([128, C], F32)
            ones_sb = pp.tile([1, 128], F32)
            scr_act = pp.tile([1, 8], F32)
            c_all = pp.tile([RT, NCC], F32)
            c_sel6 = c_all[:, 0:6]
            c_maskc = c_all[:, 6:12]
            c_ma21 = c_all[:, 12:12 + TS]
            c_mp21 = c_all[:, 12 + TS:NCC]
            # stage-0 working set
            own4 = pp.tile([RT, NJS * 4], F32)
            sxy = pp.tile([RT, NJS * 2], F32)
            rhs_all = pp.tile([RT, NJS * 12], F32)
            all_xy = pp.tile([RT, NJS * 12], F32)
            dall = pp.tile([RT, NJS * 6], F32)           # dx, d2, dist, em_minus
            dall2 = pp.tile([RT, NJS * 6], F32)          # dy, then relu scratch
            em = pp.tile([RT, NJS * 6], F32)
            esum = pp.tile([RT, NJS], F32)
            mean_sb = pp.tile([RT, NJS], F32)
            s_col = pp.tile([RT, NJS], F32)

            pts_all = pp.tile([128, NBT * CP], BF16)
            b_in = dp.tile([B, CP], BF16)
            b_out = dp.tile([256, CP], BF16)

            # -------- DMAs: consts + cdds4 + weights ahead of the pf stream
            c_gsum = pp.tile([RT, RT], F32)
            with tc.high_priority():
                nc.sync.dma_start(own4[:], cdds4)
                nc.sync.dma_start(c_gsum[:], gsum)
                nc.scalar.dma_start(c_all[:], consts)
                nc.scalar.dma_start(bias_sb[:], biasr)
                nc.sync.dma_start(
                    fcw_sb[:].rearrange("p (ht k) -> p ht k", k=2 * DL),
                    fcwT.rearrange("(ht p) k -> p ht k", p=128))
                nc.sync.dma_start(
                    clsw_sb[:].rearrange("p (ht c) -> p ht c", c=CP),
                    clsw.rearrange("(ht p) c -> p ht c", p=128))
            # ACT table prewarm for Sqrt (Exp's load stays on the chain once)
            nc.gpsimd.memset(scr_act[:], 1.0)
            nc.scalar.activation(scr_act[:], scr_act[:], ACTF.Sqrt, scale=1.0)
            nc.gpsimd.memset(ones_sb[:], 1.0)
            # wall p-part is constant: wall[:, j, 21:42] = mp21 (all j)
            wv = wall[:].rearrange("p (j f) -> p j f", f=42)
            nc.gpsimd.tensor_copy(
                ap_of(wall[:], 21, [[NJS * 42, RT], [42, NJS], [1, TS]]),
                ins_bcast(c_mp21, 1, NJS))

            # ---------------- stage 0: edge weights (3 chunks) ------------
            o4 = own4[:].rearrange("p (j f) -> p j f", f=4)
            sx2 = sxy[:].rearrange("p (j f) -> p j f", f=2)
            r12 = rhs_all[:].rearrange("p (j f) -> p j f", f=12)
            a12 = all_xy[:].rearrange("p (j f) -> p j f", f=12)
            d6 = dall[:].rearrange("p (j f) -> p j f", f=6)
            e6 = dall2[:].rearrange("p (j f) -> p j f", f=6)
            m6 = em[:].rearrange("p (j f) -> p j f", f=6)
            CH = [slice(q * JC, (q + 1) * JC) for q in range(3)]

            def sel_(q):
                return ins_bcast(c_sel6, 1, JC)

            for q, s in enumerate(CH):
                nc.vector.tensor_add(sx2[:, s, 1:2], o4[:, s, 0:1],
                                     o4[:, s, 2:3])
                nc.vector.tensor_add(sx2[:, s, 0:1], o4[:, s, 1:2],
                                     o4[:, s, 3:4])
                nc.vector.tensor_mul(r12[:, s, 0:6], sel_(q),
                                     bcast_last(sx2[:, s, 0:1], 6))
                nc.vector.tensor_mul(r12[:, s, 6:12], sel_(q),
                                     bcast_last(sx2[:, s, 1:2], 6))

            with tc.tile_pool(name="ps0", bufs=1, space="PSUM") as ps0:
                for q, s in enumerate(CH):
                    gch = ps0.tile([RT, JC * 12], F32, tag="gch", bufs=2)
                    nc.tensor.matmul(gch[:], c_gsum[:],
                                     rhs_all[:, q * JC * 12:(q + 1) * JC * 12],
                                     start=True, stop=True)
                    nc.vector.tensor_copy(
                        all_xy[:, q * JC * 12:(q + 1) * JC * 12], gch[:])
                for q, s in enumerate(CH):
                    sx_b = bcast_last(sx2[:, s, 0:1], 6)
                    sy_b = bcast_last(sx2[:, s, 1:2], 6)
                    nc.gpsimd.tensor_sub(e6[:, s, :], sy_b, a12[:, s, 6:12])
                    nc.gpsimd.tensor_mul(e6[:, s, :], e6[:, s, :], e6[:, s, :])
                    nc.vector.tensor_sub(d6[:, s, :], sx_b, a12[:, s, 0:6])
                    nc.vector.tensor_mul(d6[:, s, :], d6[:, s, :], d6[:, s, :])
                    nc.vector.tensor_add(d6[:, s, :], d6[:, s, :], e6[:, s, :])
                # single sqrt + single exp (chunking would thrash the ACT
                # function table: 1.28us reload per Sqrt<->Exp switch)
                nc.scalar.activation(dall[:], dall[:], ACTF.Sqrt, scale=0.25)
                nc.scalar.activation(dall[:], dall[:], ACTF.Exp, scale=-ALPHA)
                for q, s in enumerate(CH):
                    # em = exp * mask ; esum = sum_v em
                    nc.vector.tensor_mul(m6[:, s, :], d6[:, s, :],
                                         ins_bcast(c_maskc, 1, JC))
                    nc.vector.tensor_reduce(
                        esum[:, s], m6[:, s, :], mybir.AxisListType.X,
                        ALU.add)
                for q, s in enumerate(CH):
                    mps = ps0.tile([RT, JC], F32, tag="mps", bufs=2)
                    nc.tensor.matmul(mps[:], c_gsum[:], esum[:, s],
                                     start=True, stop=True)
                    nc.vector.tensor_copy(mean_sb[:, s], mps[:])
                for q, s in enumerate(CH):
                    # em_minus = em - mean/30 ; s' = sum_v relu(em_minus)
                    # (the 0.8 = SCALE/30 factor is folded into ma21)
                    nc.vector.scalar_tensor_tensor(
                        d6[:, s, :], ins_bcast(mean_sb[:, s], 2, 6),
                        -1.0 / 30.0, m6[:, s, :], op0=ALU.mult, op1=ALU.add)
                    nc.vector.tensor_relu(e6[:, s, :], d6[:, s, :])
                    nc.vector.tensor_reduce(s_col[:, s], e6[:, s, :],
                                            mybir.AxisListType.X, ALU.add)
                    # wall a-part: wall[:, j, 0:21] = (0.8*ma21) * s'[:, j]
                    nc.vector.tensor_mul(
                        ap_of(wall[:], q * JC * 42,
                              [[NJS * 42, RT], [42, JC], [1, TS]]),
                        ins_bcast(c_ma21, 1, JC),
                        ins_bcast(s_col[:, s], 2, TS))

            # ---------------- W2 k-slice + bias broadcast ----------------
            fS = fcw_sb[:].rearrange("p (ht k) -> p ht k", k=2 * DL)
            cS = clsw_sb[:].rearrange("p (ht c) -> p ht c", c=CP)
            w2v = w2b[:].rearrange("p (kt c) -> p kt c", c=CP)
            with tc.tile_pool(name="psw", bufs=1, space="PSUM") as psw:
                for kt in range(NKT):
                    wps = psw.tile([128, CP], F32, tag="wps", bufs=2)
                    for ht in range(NHT):
                        nc.tensor.matmul(
                            wps[:], fS[:, ht, kt * 128:(kt + 1) * 128],
                            cS[:, ht, :],
                            start=(ht == 0), stop=(ht == NHT - 1))
                    nc.vector.tensor_copy(w2v[:, kt, :], wps[:])
                bps = psw.tile([128, C], F32, tag="bps")
                nc.tensor.matmul(bps[:], ones_sb[:], bias_sb[:], start=True,
                                 stop=True)
                nc.vector.tensor_copy(bias_bc[:], bps[:])

            # ---------------- stage 1 + interleaved stage 2 ----------------
            xv = xT[:].rearrange("p (kt b) -> p kt b", b=B)
            lo, nl = LO, NL
            state = {"bt": 0, "ev": 0, "pt": None}
            with tc.tile_pool(name="pfp", bufs=7) as pfp, \
                 tc.tile_pool(name="ps1", bufs=1, space="PSUM") as ps1, \
                 tc.tile_pool(name="ps2", bufs=1, space="PSUM") as ps2:

                def do_btile(t):
                    ops = ps2.tile([128, CP], F32, tag="ops", bufs=2)
                    for kt in range(NKT):
                        nc.tensor.matmul(
                            ops[:],
                            ap_of(xT[:], kt * B + 128 * t,
                                  [[NKT * B, 128], [1, 128]]),
                            w2v[:, kt, :],
                            start=(kt == 0), stop=(kt == NKT - 1))
                    if t >= NBT - 4:
                        nc.vector.tensor_copy(
                            pts_all[:, t * CP:(t + 1) * CP], ops[:])
                    else:
                        nc.scalar.copy(pts_all[:, t * CP:(t + 1) * CP],
                                       ops[:])

                def post():
                    # shard rows (h*128+p) -> SBUF [p, h, c]; add bias; store
                    pb = pp.tile([128, 2 * CP], BF16, name="pb")
                    pfq = pp.tile([128, 2 * C], F32, name="pfq")
                    nc.sync.dma_start(
                        pb[:].rearrange("p (h c) -> p h c", c=CP),
                        b_out.opt().rearrange("(h p) c -> p h c", p=128))
                    nc.vector.tensor_add(
                        pfq[:].rearrange("p (h c) -> p h c", c=C),
                        ap_of(pb[:], 0, [[2 * CP, 128], [CP, 2], [1, C]]),
                        ins_bcast(bias_bc[:], 1, 2))
                    nc.sync.dma_start(
                        out.rearrange("(h p) c -> p h c", p=128),
                        pfq[:].rearrange("p (h c) -> p h c", c=C))

                def do_tile(j, pft_ap):
                    last = j == NJ - 1 and lo > 0
                    rt = 6 * nl if last else RT
                    ns = nl if last else TS
                    if last:
                        rhs_w = ap_of(wall[:], (NJS - 1) * 42,
                                      [[NJS * 42, rt], [21, 2], [1, ns]])
                    else:
                        rhs_w = wv[:, j, :]
                    psA = ps1.tile([128, 84], F32, tag="psA", bufs=6)
                    for db in range(2):
                        nc.tensor.matmul(
                            psA[:, db * 2 * ns:(db + 1) * 2 * ns],
                            pft_ap[0:rt, db * 128:(db + 1) * 128],
                            rhs_w, start=True, stop=True)
                    c0 = b0s[j] + lo if last else b0s[j]
                    # one 4D copy: psA[p, db, h, s] -> xT[p, (2h+db)*B + c0+s]
                    src = ap_of(psA[:], 0,
                                [[84, 128], [ns, 2], [2 * ns, 2], [1, ns]])
                    dst = ap_of(xT[:], c0,
                                [[NKT * B, 128], [2 * B, 2], [B, 2], [1, ns]])
                    if state["ev"] % 2 == 0:
                        nc.vector.tensor_copy(dst, src)
                    else:
                        nc.scalar.copy(dst, src)
                    state["ev"] += 1
                    if last:
                        return   # tail runs first; it must not advance bt
                    cov = TS * (j + 1)   # samples covered by regular tiles
                    while state["bt"] < NBT and (
                            128 * (state["bt"] + 1) <= cov
                            or (state["bt"] == NBT - 1 and cov >= B - NL)):
                        do_btile(state["bt"])
                        state["bt"] += 1

                pf_tl = pfp.tile([6 * NL, DL], F32, tag="pftail", bufs=1)
                nc.sync.dma_start(pf_tl[:], pf[(B - NL) * 6:B * 6, :])
                do_tile(NJ - 1, pf_tl[:])
                for si, (j0, J) in enumerate(sts):
                    pf_st = pfp.tile([RT, STJ * DL], F32, tag="pf", bufs=7)
                    nc.sync.dma_start(
                        pf_st[:, 0:J * DL].rearrange(
                            "p (jj d) -> p jj d", d=DL),
                        ap_of(pf, j0 * RT * DL,
                              [[DL, RT], [RT * DL, J], [1, DL]]))
                    for jj in range(J):
                        do_tile(j0 + jj, pf_st[:, jj * DL:(jj + 1) * DL])
                while state["bt"] < NBT:
                    do_btile(state["bt"])
                    state["bt"] += 1
                # partials -> b_in: one bulk DMA for slots 0..13 plus two
                # solos, all on the (now idle) SP HWDGE queue
                nc.sync.dma_start(
                    ap_of(b_in.opt(), 0,
                          [[CP, 128], [128 * CP, NBT - 2], [1, CP]]),
                    ap_of(pts_all[:], 0,
                          [[NBT * CP, 128], [CP, NBT - 2], [1, CP]]))
                for t in (NBT - 2, NBT - 1):
                    nc.sync.dma_start(
                        ap_of(b_in.opt(), t * 128 * CP, [[CP, 128], [1, CP]]),
                        pts_all[:, t * CP:(t + 1) * CP])
                nc.gpsimd.collective_compute(
                    "ReduceScatter", ALU.add,
                    replica_groups=[list(range(n_cores))],
                    ins=[b_in.opt()], outs=[b_out.opt()])
                post()
    nc.compile()
    return nc


def make_host_inputs(part_feats, cdds, fc_w, fc_b, cls_w, cls_b, n_cores=8):
    """Shard + prepare per-core in_maps from full inputs."""
    B = part_feats.shape[0]
    p = np.arange(RT)
    maskc = (p[:, None] % 6 != np.arange(6)[None, :]).astype(np.float32)
    sel = (p[:, None] % 6 == np.arange(6)[None, :]).astype(np.float32)
    gs = (p[:, None] // 6 == p[None, :] // 6).astype(np.float32)
    ma = np.zeros((RT, TS), np.float32)
    ma[p, p // 6] = 0.8                  # SCALE/30 folded in
    mp = np.zeros((RT, TS), np.float32)
    mp[p, p // 6] = 1.0 / 6.0
    bias = (fc_b @ cls_w + cls_b).reshape(1, C).astype(np.float32)
    # cdds4[p=(t,u), j, 0:4] = cdds[b0s[j] + t, 1+6u : 5+6u]; tail column
    # (j = NJ) = the last NL samples at partitions 0:6*NL, zero-padded.
    NJ = -(-B // TS)
    LO = NJ * TS - B
    NL = TS - LO
    NJS = NJ + (1 if LO else 0)
    b0s = np.array([TS * j for j in range(NJ - 1)] + [B - TS])
    cd = cdds.reshape(B, 6, 6)[:, :, 1:5]                # (B, u, 4)
    t_i, u_i = p // 6, p % 6
    cdds4 = np.zeros((RT, NJS, 4), np.float32)
    cdds4[:, 0:NJ, :] = cd[b0s[None, :] + t_i[:, None], u_i[:, None], :]
    if LO:
        cdds4[0:6 * NL, NJ, :] = cd[B - NL:B].reshape(6 * NL, 4)
    cdds4 = np.ascontiguousarray(cdds4.reshape(RT, NJS * 4))
    bf = ml_dtypes.bfloat16
    f8 = ml_dtypes.float8_e4m3fn
    clsw_pad = np.zeros((H, CP), np.float32)
    clsw_pad[:, 0:C] = cls_w
    clsw_pad = clsw_pad.astype(bf)
    in_maps = []
    for c in range(n_cores):
        rows = np.r_[c * DL:(c + 1) * DL, D + c * DL:D + (c + 1) * DL]
        in_maps.append({
            "pf": np.ascontiguousarray(
                part_feats[:, :, c * DL:(c + 1) * DL].reshape(B * NN, DL)),
            "cdds4": cdds4,
            "fcwT": np.ascontiguousarray(fc_w[rows, :].T).astype(bf),
            "clsw": clsw_pad,
            "biasr": bias,
            "consts": np.ascontiguousarray(
                np.concatenate([sel, maskc, ma, mp], axis=1)),
            "gsum": gs,
        })
    return in_maps


_NC_CACHE = {}


def kernel(part_feats, cdds, fc_w, fc_b, cls_w, cls_b):
    part_feats = np.ascontiguousarray(part_feats, dtype=np.float32)
    cdds = np.ascontiguousarray(cdds, dtype=np.float32)
    fc_w = np.ascontiguousarray(fc_w, dtype=np.float32)
    fc_b = np.ascontiguousarray(fc_b, dtype=np.float32)
    cls_w = np.ascontiguousarray(cls_w, dtype=np.float32)
    cls_b = np.ascontiguousarray(cls_b, dtype=np.float32)
    B = part_feats.shape[0]
    if "nc" not in _NC_CACHE:
        _NC_CACHE["nc"] = build_nc(n_cores=N_CORES)
    nc = _NC_CACHE["nc"]
    in_maps = make_host_inputs(part_feats, cdds, fc_w, fc_b, cls_w, cls_b,
                               n_cores=N_CORES)
    # First execution after compile has been observed to produce bad output
    # intermittently (runtime warmup); run once to warm up, then take the
    # second execution's result.
    bass_utils.run_bass_kernel_spmd(
        nc, in_maps, core_ids=list(range(N_CORES)))
    res = bass_utils.run_bass_kernel_spmd(
        nc, in_maps, core_ids=list(range(N_CORES)))
    # core c's "out" = samples [256c, 256(c+1))
    return np.concatenate([res.results[c]["out"] for c in range(N_CORES)],
                          axis=0)
